# revision 8
# baseline (speedup 1.0000x reference)
"""Trainium2 Bass kernel for dual cross-attention (CotSR block).

Problem: two cross-attentions between x1, x2 [B=4, C=512, H=W=64].
  q1 = wq1@x1, k2 = wk2@x2, v2 = wv2@x2 ; att1 = softmax(q1^T k2) over keys
  out1 = x1 + gamma1 * (v2 @ att1^T)   (and symmetrically for out2)

Sharding: 8 independent (batch, direction) jobs -> one per NeuronCore.

Per-core dataflow (N = 4096 tokens, DQ = 64, C = 512), v4:
  - x arrives as fp8e4 (gpsimd DMA casts f32->fp8 in flight); xq piece 0
    first so Q0 unblocks early.
  - All projections are fp8 DoubleRow matmuls (contraction 256/pass);
    weights host-transposed, x16-scaled into fp8e4; evictions scale 1/16.
    Q/K weights host-duplicated so dq=64 lands on partitions 0-63 AND
    64-127 (enables 2x row-tiled ST matmuls).
  - K blocks and VT tiles are software-pipelined INTO qblock 0's pair loop
    (pair t only needs K/VT keytiles 2t,2t+1), so attention starts as soon
    as the first xkv DMA piece lands instead of after all projections.
  - Attention per 512-query block, over 16 key-tile PAIRS (256 keys each):
      ST:  two concurrent row-tiled bf16 matmuls (rows 0-63 / 64-127)
      PT:  exp(ST - 7) -> fp8e5 pair tile [128, 2, 512]  (global shift is
           exact for softmax: numerator and denominator both scale e^-7)
      RS:  1 DoubleRow matmul, all-ones lhsT -> BROADCAST row-sum [128,512]
      PV:  4 DoubleRow fp8 matmuls (contraction 256) accumulate [128c, 512q]
  - PSUM 5-bank accumulator rotation: qblock qb+1's first PV reuses the
    bank freed by qb's recip (earliest), later PVs the banks freed by the
    per-cc normalization muls -> near-zero qblock seam.
  PSUM: 5 banks rotating O/RS + 3 banks ST ring = 8.
"""

import numpy as np

import concourse.bass as bass
import concourse.mybir as mybir
import concourse.tile as tile
from concourse import bacc
import concourse.bass_utils as _bu

# walrus's --enable-ldw-opt=false serializes every LDWEIGHTS with its MATMUL
# (measured 379 ns/MM vs ~215 warm); enable background-weight-buffer overlap.
_orig_run_command = _bu.run_command


def _patched_run_command(argv, **kw):
    argv = ["--enable-ldw-opt=true" if a == "--enable-ldw-opt=false" else a
            for a in argv]
    return _orig_run_command(argv, **kw)


_bu.run_command = _patched_run_command
from concourse.bass_utils import run_bass_kernel_spmd
from concourse._compat import with_exitstack
from contextlib import ExitStack

F32 = mybir.dt.float32
BF16 = mybir.dt.bfloat16
F8E4 = mybir.dt.float8e4
F8E5 = mybir.dt.float8e5
AF = mybir.ActivationFunctionType
ALU = mybir.AluOpType
DR = mybir.MatmulPerfMode.DoubleRow
ts = bass.ts

B, C, H, W = 4, 512, 64, 64
N = H * W          # 4096
DQ = 64
P = 128
QB = 512           # query block (free dim of ST / moving operand)
NQB = N // QB      # 8 query blocks
NKT = N // P       # 32 key tiles
NP = NKT // 2      # 16 key-tile pairs
NCC = C // P       # 4 channel chunks
SHIFT = 7.0        # global logit shift before exp (softmax-invariant)
WS = 16.0          # fp8 weight scale (undone at psum eviction)


@with_exitstack
def _body(ctx: ExitStack, tc: "tile.TileContext", io: dict):
    nc = tc.nc
    xq_d, xkv_d, wq_d, wk_d, wv_d = io["xq"], io["xkv"], io["wq"], io["wk"], io["wv"]
    bq_d, bk_d, bv_d, gamma_d, out_d = io["bq"], io["bk"], io["bv"], io["gamma"], io["out"]

    const = ctx.enter_context(tc.tile_pool(name="const", bufs=1))
    persist = ctx.enter_context(tc.tile_pool(name="persist", bufs=1))
    wpool = ctx.enter_context(tc.tile_pool(name="wpool", bufs=1))
    stage = ctx.enter_context(tc.tile_pool(name="stage", bufs=3))
    ptp = ctx.enter_context(tc.tile_pool(name="ptp", bufs=3))
    dvp = ctx.enter_context(tc.tile_pool(name="dvp", bufs=3))
    pacc = ctx.enter_context(tc.tile_pool(name="pacc", bufs=1, space="PSUM"))
    pst = ctx.enter_context(tc.tile_pool(name="pst", bufs=3, space="PSUM"))

    # ---- constants ----
    ones_dr = const.tile([P, 2, P], F8E4, tag="ones_dr", name="ones_dr")
    nc.vector.memset(ones_dr, 1.0)
    ones_row_bf = const.tile([1, P], BF16, tag="ones_row_bf", name="ones_row_bf")
    nc.vector.memset(ones_row_bf, 1.0)
    negshift = const.tile([P, 1], F32, tag="negshift", name="negshift")
    nc.vector.memset(negshift, -SHIFT)

    # ---- small inputs (biases host-duplicated to [128,1]) ----
    bq_sb = const.tile([P, 1], F32, tag="bq", name="bq_sb")
    nc.sync.dma_start(bq_sb, bq_d)
    bk_sb = const.tile([P, 1], F32, tag="bk", name="bk_sb")
    nc.sync.dma_start(bk_sb, bk_d)
    bv_sb = const.tile([1, C], F32, tag="bv", name="bv_sb")
    nc.sync.dma_start(bv_sb, bv_d)
    bv_bf = const.tile([1, C], BF16, tag="bvbf", name="bv_bf")
    nc.vector.tensor_copy(bv_bf, bv_sb)
    gamma_b = const.tile([P, 1], F32, tag="gamma_b", name="gamma_b")
    nc.sync.dma_start(gamma_b, gamma_d)

    # bv broadcast to all partitions once: [128, C] bf16
    bvb_ps = pst.tile([P, C], F32, tag="st", name="bvb_ps")
    nc.tensor.matmul(bvb_ps, ones_row_bf, bv_bf, start=True, stop=True)
    bv_bcast = const.tile([P, C], BF16, tag="bv_bcast", name="bv_bcast")
    nc.vector.tensor_copy(bv_bcast, bvb_ps)

    # ---- weights: host pre-transposed (+ dq-duplicated for Q/K); stage f32
    # then x16-scale into fp8e4 pair-sliceable tiles ----
    wq8 = wpool.tile([P, NCC, P], F8E4, tag="wq8", name="wq8")
    wk8 = wpool.tile([P, NCC, P], F8E4, tag="wk8", name="wk8")
    for j in range(NCC):
        for (src_d, dst) in ((wq_d, wq8), (wk_d, wk8)):
            wst = stage.tile([P, P], F32, tag="w_stage", name="w_st")
            nc.sync.dma_start(wst, src_d[ts(j, P), :])
            nc.vector.tensor_scalar_mul(dst[:, j, :], wst, WS)
    wv8 = wpool.tile([P, NCC, C], F8E4, tag="wv8", name="wv8")
    for j in range(NCC):
        wst2 = stage.tile([P, C], F32, tag="w_stage2", name="w_st2")
        nc.sync.dma_start(wst2, wv_d[ts(j, P), :])
        nc.vector.tensor_scalar_mul(wv8[:, j, :], wst2, WS)

    # ---- x resident fp8e4; gpsimd DMA casts f32->fp8 in flight.
    # xq piece 0 first (unblocks Q0), then all xkv, then the rest of xq.
    xq8 = persist.tile([P, NCC, N], F8E4, tag="xq8", name="xq8")
    xkv8 = persist.tile([P, NCC, N], F8E4, tag="xkv8", name="xkv8")
    for cc in range(NCC):
        nc.gpsimd.dma_start(xq8[:, cc, ts(0, 1024)], xq_d[ts(cc, P), ts(0, 1024)])
    for h in range(4):  # 1024-col pieces
        for cc in range(NCC):
            nc.gpsimd.dma_start(xkv8[:, cc, ts(h, 1024)],
                                xkv_d[ts(cc, P), ts(h, 1024)])
    for h in range(1, 4):
        for cc in range(NCC):
            nc.gpsimd.dma_start(xq8[:, cc, ts(h, 1024)],
                                xq_d[ts(cc, P), ts(h, 1024)])

    # ---- projection helpers (fp8 DoubleRow, contraction 2x128/pass) ----
    # Q/K land duplicated on both partition halves: [128(2x dq), N]
    Q_sb = persist.tile([P, N], BF16, tag="Q", name="Q_sb")
    K_sb = persist.tile([P, N], BF16, tag="K", name="K_sb")
    VT_sb = persist.tile([P, NKT, C], F8E4, tag="VT", name="VT_sb")

    def emit_k(nb, pool, tag):
        kp = pool.tile([P, QB], F32, tag=tag, name="k_ps")
        for m in range(NCC // 2):
            nc.tensor.matmul(kp, wk8[:, 2 * m:2 * m + 2, :],
                             xkv8[:, 2 * m:2 * m + 2, ts(nb, QB)],
                             start=(m == 0), stop=(m == 1), perf_mode=DR)
        nc.scalar.activation(K_sb[:, ts(nb, QB)], kp, AF.Identity,
                             bias=bk_sb, scale=1.0 / WS)

    def emit_q(nb, pool, tag):
        qp = pool.tile([P, QB], F32, tag=tag, name="q_ps")
        for m in range(NCC // 2):
            nc.tensor.matmul(qp, wq8[:, 2 * m:2 * m + 2, :],
                             xq8[:, 2 * m:2 * m + 2, ts(nb, QB)],
                             start=(m == 0), stop=(m == 1), perf_mode=DR)
        nc.scalar.activation(Q_sb[:, ts(nb, QB)], qp, AF.Identity,
                             bias=bq_sb, scale=1.0 / WS)

    def emit_vt(nt):
        vp = pst.tile([P, C], F32, tag="st", name="v_ps")
        for m in range(NCC // 2):
            nc.tensor.matmul(vp, xkv8[:, 2 * m:2 * m + 2, ts(nt, P)],
                             wv8[:, 2 * m:2 * m + 2, :],
                             start=(m == 0), stop=(m == 1), perf_mode=DR)
        nc.vector.scalar_tensor_tensor(VT_sb[:, nt, :], vp, 1.0 / WS, bv_bcast,
                                       op0=ALU.mult, op1=ALU.add)

    # K0/K1 (gated by xkv piece 0) and Q0/Q1 (xq piece 0) upfront in the
    # not-yet-rotating accumulator banks; everything else pipelines into
    # qblock 0's pair loop below.
    emit_k(0, pacc, "a0")
    emit_k(1, pacc, "a1")
    emit_q(0, pacc, "a2")
    emit_q(1, pacc, "a3")
    emit_vt(0)
    emit_vt(1)

    # ---- attention main loop ----
    def emit_st_exp(qb, t):
        """ST pair t (two row-tiled concurrent matmuls) + exp -> fp8e5 pair."""
        st0 = pst.tile([P, QB], F32, tag="st", name="st_ps")
        nc.tensor.matmul(st0, K_sb[0:DQ, ts(2 * t, P)],
                         Q_sb[0:DQ, ts(qb, QB)], start=True, stop=True)
        st1 = pst.tile([P, QB], F32, tag="st", name="st_ps")
        nc.tensor.matmul(st1, K_sb[DQ:P, ts(2 * t + 1, P)],
                         Q_sb[DQ:P, ts(qb, QB)], start=True, stop=True)
        pt = ptp.tile([P, 2, QB], F8E5, tag="pt", name="pt_sb", bufs=8)
        nc.scalar.activation(pt[:, 0, :], st0, AF.Exp, bias=negshift)
        nc.scalar.activation(pt[:, 1, :], st1, AF.Exp, bias=negshift)
        return pt

    def tail(qb, o_ps, rs_ps, xrs):
        # recip first (frees the rs bank), then the four normalization muls
        # straight from the o psum banks (each frees its bank in rotation
        # order), then residual-add + store.
        recip_b = dvp.tile([P, QB], F32, tag="recip_b", name="recip_b")
        nc.vector.reciprocal(recip_b, rs_ps)
        t1s = []
        for cc in range(NCC):
            t1 = dvp.tile([P, QB], F32, tag=f"t1_{cc}", name="t1", bufs=2)
            nc.vector.tensor_mul(t1, o_ps[cc], recip_b)
            t1s.append(t1)
        for cc in range(NCC):
            og = dvp.tile([P, QB], F32, tag="og", name="og", bufs=2)
            nc.vector.scalar_tensor_tensor(og, t1s[cc], gamma_b, xrs[cc],
                                           op0=ALU.mult, op1=ALU.add)
            nc.sync.dma_start(out_d[ts(cc, P), ts(qb, QB)], og)

    pt_next = emit_st_exp(0, 0)
    for qb in range(NQB):
        # bank-role rotation: o'0 takes the bank qb-1's recip freed, o'1..3
        # the banks the per-cc muls freed, rs' the bank the last mul freed.
        o_ps = [pacc.tile([P, QB], F32, tag=f"a{(cc - qb) % 5}",
                          name=f"o_ps{cc}") for cc in range(NCC)]
        rs_ps = pacc.tile([P, QB], F32, tag=f"a{(4 - qb) % 5}", name="rs_ps")
        # prefetch this qblock's residual slices (consumed by the tail)
        xrs = []
        for cc in range(NCC):
            xr = stage.tile([P, QB], F32, tag=f"xres{cc}", name="x_res", bufs=2)
            nc.sync.dma_start(xr, xq_d[ts(cc, P), ts(qb, QB)])
            xrs.append(xr)
        for t in range(NP):
            if qb == 0:
                # software-pipelined projections: VT pair one pair ahead of
                # its consumer, K block two pairs ahead, all through the st
                # psum ring.
                if t < NP - 1:
                    emit_vt(2 * t + 2)
                    emit_vt(2 * t + 3)
                if t % 2 == 1 and t < 13:
                    emit_k((t + 3) // 2, pst, "st")
            pt = pt_next
            g = qb * NP + t + 1
            if g < NQB * NP:
                pt_next = emit_st_exp(g // NP, g % NP)
            if t == NP - 1:
                # rs first: its stop unblocks the recip early
                nc.tensor.matmul(rs_ps, ones_dr, pt,
                                 start=False, stop=True, perf_mode=DR)
            for cc in range(NCC):
                nc.tensor.matmul(o_ps[cc], VT_sb[:, 2 * t:2 * t + 2, ts(cc, P)],
                                 pt, start=(t == 0), stop=(t == NP - 1),
                                 perf_mode=DR)
            if t == 0:
                nc.tensor.matmul(rs_ps, ones_dr, pt,
                                 start=True, stop=False, perf_mode=DR)
            elif t < NP - 1:
                nc.tensor.matmul(rs_ps, ones_dr, pt,
                                 start=False, stop=False, perf_mode=DR)
            if t == 8 and qb + 2 < NQB:
                emit_q(qb + 2, pst, "st")
        tail(qb, o_ps, rs_ps, xrs)


_NC_CACHE = {}


def _fuse_ldweights(nc):
    """Re-fuse Tile's split LDWEIGHTS+MATMUL pairs into self-loading matmuls
    so walrus's ldw-opt (background weight buffer) can overlap weight loads
    with in-flight matmuls."""
    for b in nc.m.functions[0].blocks:
        out = []
        pending = None
        for i in b.instructions:
            tn = type(i).__name__
            if tn == "InstLdweights":
                assert pending is None, "back-to-back ldweights"
                pending = i
                continue
            if tn == "InstMatmult" and pending is not None:
                i.ldweights = True
                si = pending.sync_info
                if si is not None and (si.on_wait or si.on_update):
                    if i.sync_info is None:
                        i.sync_info = mybir.SyncInfo(on_wait=[], on_update=[])
                    i.sync_info.on_wait = list(si.on_wait) + list(i.sync_info.on_wait)
                    i.sync_info.on_update = (list(si.on_update)
                                             + list(i.sync_info.on_update))
                pending = None
            out.append(i)
        assert pending is None, "trailing ldweights without matmul"
        b.instructions[:] = out


def _build():
    if "nc" in _NC_CACHE:
        return _NC_CACHE["nc"]
    nc = bacc.Bacc("TRN2", target_bir_lowering=False, debug=False, num_devices=8)
    io = {
        "xq": nc.dram_tensor("xq", [C, N], F32, kind="ExternalInput").ap(),
        "xkv": nc.dram_tensor("xkv", [C, N], F32, kind="ExternalInput").ap(),
        "wq": nc.dram_tensor("wq", [C, P], F32, kind="ExternalInput").ap(),
        "wk": nc.dram_tensor("wk", [C, P], F32, kind="ExternalInput").ap(),
        "wv": nc.dram_tensor("wv", [C, C], F32, kind="ExternalInput").ap(),
        "bq": nc.dram_tensor("bq", [P, 1], F32, kind="ExternalInput").ap(),
        "bk": nc.dram_tensor("bk", [P, 1], F32, kind="ExternalInput").ap(),
        "bv": nc.dram_tensor("bv", [1, C], F32, kind="ExternalInput").ap(),
        "gamma": nc.dram_tensor("gamma", [P, 1], F32, kind="ExternalInput").ap(),
        "out": nc.dram_tensor("out", [C, N], F32, kind="ExternalOutput").ap(),
    }
    with tile.TileContext(nc) as tc:
        _body(tc, io)
    _fuse_ldweights(nc)
    nc.compile()
    _NC_CACHE["nc"] = nc
    return nc


def make_in_maps(x1, x2, wq1, bq1, wk1, bk1, wv1, bv1,
                 wq2, bq2, wk2, bk2, wv2, bv2, gamma1, gamma2):
    """Returns the 8 per-core input dicts. Cores 0-3: out1[b]; 4-7: out2[b]."""
    f = np.ascontiguousarray
    x1f = np.asarray(x1, np.float32).reshape(B, C, N)
    x2f = np.asarray(x2, np.float32).reshape(B, C, N)

    def wdup(w):  # [DQ, C] -> [C, 2*DQ] (transposed, duplicated)
        wt = np.asarray(w, np.float32).T
        return f(np.concatenate([wt, wt], axis=1))

    def bdup(b):  # [DQ] -> [128, 1]
        bb = np.asarray(b, np.float32).reshape(DQ, 1)
        return f(np.concatenate([bb, bb], axis=0))

    maps = []
    for b in range(B):
        maps.append({
            "xq": f(x1f[b]), "xkv": f(x2f[b]),
            "wq": wdup(wq1), "wk": wdup(wk2),
            "wv": f(np.asarray(wv2, np.float32).T),
            "bq": bdup(bq1), "bk": bdup(bk2),
            "bv": f(np.asarray(bv2, np.float32).reshape(1, C)),
            "gamma": f(np.tile(np.asarray(gamma1, np.float32).reshape(1, 1), (P, 1))),
        })
    for b in range(B):
        maps.append({
            "xq": f(x2f[b]), "xkv": f(x1f[b]),
            "wq": wdup(wq2), "wk": wdup(wk1),
            "wv": f(np.asarray(wv1, np.float32).T),
            "bq": bdup(bq2), "bk": bdup(bk1),
            "bv": f(np.asarray(bv1, np.float32).reshape(1, C)),
            "gamma": f(np.tile(np.asarray(gamma2, np.float32).reshape(1, 1), (P, 1))),
        })
    return maps


def kernel(**inputs):
    nc = _build()
    in_maps = make_in_maps(**inputs)
    res = run_bass_kernel_spmd(nc, in_maps, list(range(8))).results
    out1 = np.stack([res[b]["out"].reshape(C, H, W) for b in range(B)])
    out2 = np.stack([res[B + b]["out"].reshape(C, H, W) for b in range(B)])
    return out1, out2


# revision 10
# speedup vs baseline: 1.0271x; 1.0271x over previous
"""Trainium2 Bass kernel for dual cross-attention (CotSR block).

Problem: two cross-attentions between x1, x2 [B=4, C=512, H=W=64].
  q1 = wq1@x1, k2 = wk2@x2, v2 = wv2@x2 ; att1 = softmax(q1^T k2) over keys
  out1 = x1 + gamma1 * (v2 @ att1^T)   (and symmetrically for out2)

Sharding: 8 independent (batch, direction) jobs -> one per NeuronCore.

Per-core dataflow (N = 4096 tokens, DQ = 64, C = 512), v4:
  - x arrives as fp8e4 (gpsimd DMA casts f32->fp8 in flight); xq piece 0
    first so Q0 unblocks early.
  - All projections are fp8 DoubleRow matmuls (contraction 256/pass);
    weights host-transposed, x16-scaled into fp8e4; evictions scale 1/16.
    Q/K weights host-duplicated so dq=64 lands on partitions 0-63 AND
    64-127 (enables 2x row-tiled ST matmuls).
  - K blocks and VT tiles are software-pipelined INTO qblock 0's pair loop
    (pair t only needs K/VT keytiles 2t,2t+1), so attention starts as soon
    as the first xkv DMA piece lands instead of after all projections.
  - Attention per 512-query block, over 16 key-tile PAIRS (256 keys each):
      ST:  two concurrent row-tiled bf16 matmuls (rows 0-63 / 64-127)
      PT:  exp(ST - 7) -> fp8e5 pair tile [128, 2, 512]  (global shift is
           exact for softmax: numerator and denominator both scale e^-7)
      RS:  1 DoubleRow matmul, all-ones lhsT -> BROADCAST row-sum [128,512]
      PV:  4 DoubleRow fp8 matmuls (contraction 256) accumulate [128c, 512q]
  - PSUM 5-bank accumulator rotation: qblock qb+1's first PV reuses the
    bank freed by qb's recip (earliest), later PVs the banks freed by the
    per-cc normalization muls -> near-zero qblock seam.
  PSUM: 5 banks rotating O/RS + 3 banks ST ring = 8.
"""

import numpy as np

import concourse.bass as bass
import concourse.mybir as mybir
import concourse.tile as tile
from concourse import bacc
import concourse.bass_utils as _bu

# walrus's --enable-ldw-opt=false serializes every LDWEIGHTS with its MATMUL
# (measured 379 ns/MM vs ~215 warm); enable background-weight-buffer overlap.
_orig_run_command = _bu.run_command


def _patched_run_command(argv, **kw):
    argv = ["--enable-ldw-opt=true" if a == "--enable-ldw-opt=false" else a
            for a in argv]
    return _orig_run_command(argv, **kw)


_bu.run_command = _patched_run_command
from concourse.bass_utils import run_bass_kernel_spmd
from concourse._compat import with_exitstack
from contextlib import ExitStack

F32 = mybir.dt.float32
BF16 = mybir.dt.bfloat16
F8E4 = mybir.dt.float8e4
F8E5 = mybir.dt.float8e5
AF = mybir.ActivationFunctionType
ALU = mybir.AluOpType
DR = mybir.MatmulPerfMode.DoubleRow
ts = bass.ts

B, C, H, W = 4, 512, 64, 64
N = H * W          # 4096
DQ = 64
P = 128
QB = 512           # query block (free dim of ST / moving operand)
NQB = N // QB      # 8 query blocks
NKT = N // P       # 32 key tiles
NP = NKT // 2      # 16 key-tile pairs
NCC = C // P       # 4 channel chunks
SHIFT = 7.0        # global logit shift before exp (softmax-invariant)
WS = 16.0          # fp8 weight scale (undone at psum eviction)


@with_exitstack
def _body(ctx: ExitStack, tc: "tile.TileContext", io: dict):
    nc = tc.nc
    xq_d, xkv_d, wq_d, wk_d, wv_d = io["xq"], io["xkv"], io["wq"], io["wk"], io["wv"]
    bq_d, bk_d, bv_d, gamma_d, out_d = io["bq"], io["bk"], io["bv"], io["gamma"], io["out"]

    const = ctx.enter_context(tc.tile_pool(name="const", bufs=1))
    persist = ctx.enter_context(tc.tile_pool(name="persist", bufs=1))
    wpool = ctx.enter_context(tc.tile_pool(name="wpool", bufs=1))
    stage = ctx.enter_context(tc.tile_pool(name="stage", bufs=3))
    ptp = ctx.enter_context(tc.tile_pool(name="ptp", bufs=3))
    dvp = ctx.enter_context(tc.tile_pool(name="dvp", bufs=3))
    pacc = ctx.enter_context(tc.tile_pool(name="pacc", bufs=1, space="PSUM"))
    pst = ctx.enter_context(tc.tile_pool(name="pst", bufs=3, space="PSUM"))

    # ---- constants ----
    ones_dr = const.tile([P, 2, P], F8E4, tag="ones_dr", name="ones_dr")
    nc.vector.memset(ones_dr, 1.0)
    ones_row_bf = const.tile([1, P], BF16, tag="ones_row_bf", name="ones_row_bf")
    nc.vector.memset(ones_row_bf, 1.0)
    negshift = const.tile([P, 1], F32, tag="negshift", name="negshift")
    nc.vector.memset(negshift, -SHIFT)

    # ---- small inputs (biases host-duplicated to [128,1]) ----
    bq_sb = const.tile([P, 1], F32, tag="bq", name="bq_sb")
    nc.sync.dma_start(bq_sb, bq_d)
    bk_sb = const.tile([P, 1], F32, tag="bk", name="bk_sb")
    nc.sync.dma_start(bk_sb, bk_d)
    bv_sb = const.tile([1, C], F32, tag="bv", name="bv_sb")
    nc.sync.dma_start(bv_sb, bv_d)
    bv_bf = const.tile([1, C], BF16, tag="bvbf", name="bv_bf")
    nc.vector.tensor_copy(bv_bf, bv_sb)
    gamma_b = const.tile([P, 1], F32, tag="gamma_b", name="gamma_b")
    nc.sync.dma_start(gamma_b, gamma_d)

    # bv broadcast to all partitions once: [128, C] bf16
    bvb_ps = pst.tile([P, C], F32, tag="st", name="bvb_ps")
    nc.tensor.matmul(bvb_ps, ones_row_bf, bv_bf, start=True, stop=True)
    bv_bcast = const.tile([P, C], BF16, tag="bv_bcast", name="bv_bcast")
    nc.vector.tensor_copy(bv_bcast, bvb_ps)

    # ---- weights: host pre-transposed (+ dq-duplicated for Q/K); stage f32
    # then x16-scale into fp8e4 pair-sliceable tiles ----
    wq8 = wpool.tile([P, NCC, P], F8E4, tag="wq8", name="wq8")
    wk8 = wpool.tile([P, NCC, P], F8E4, tag="wk8", name="wk8")
    for j in range(NCC):
        for (src_d, dst) in ((wq_d, wq8), (wk_d, wk8)):
            wst = stage.tile([P, P], F32, tag="w_stage", name="w_st")
            nc.sync.dma_start(wst, src_d[ts(j, P), :])
            nc.vector.tensor_scalar_mul(dst[:, j, :], wst, WS)
    wv8 = wpool.tile([P, NCC, C], F8E4, tag="wv8", name="wv8")
    for j in range(NCC):
        wst2 = stage.tile([P, C], F32, tag="w_stage2", name="w_st2")
        nc.sync.dma_start(wst2, wv_d[ts(j, P), :])
        nc.vector.tensor_scalar_mul(wv8[:, j, :], wst2, WS)

    # ---- x resident fp8e4; gpsimd DMA casts f32->fp8 in flight.
    # xq piece 0 first (unblocks Q0), then all xkv, then the rest of xq.
    xq8 = persist.tile([P, NCC, N], F8E4, tag="xq8", name="xq8")
    xkv8 = persist.tile([P, NCC, N], F8E4, tag="xkv8", name="xkv8")
    for cc in range(NCC):
        nc.gpsimd.dma_start(xq8[:, cc, ts(0, 1024)], xq_d[ts(cc, P), ts(0, 1024)])
    for h in range(4):  # 1024-col pieces
        for cc in range(NCC):
            nc.gpsimd.dma_start(xkv8[:, cc, ts(h, 1024)],
                                xkv_d[ts(cc, P), ts(h, 1024)])
    for h in range(1, 4):
        for cc in range(NCC):
            nc.gpsimd.dma_start(xq8[:, cc, ts(h, 1024)],
                                xq_d[ts(cc, P), ts(h, 1024)])

    # ---- projection helpers (fp8 DoubleRow, contraction 2x128/pass) ----
    # Q/K land duplicated on both partition halves: [128(2x dq), N]
    Q_sb = persist.tile([P, N], BF16, tag="Q", name="Q_sb")
    K_sb = persist.tile([P, N], BF16, tag="K", name="K_sb")
    VT_sb = persist.tile([P, NKT, C], F8E4, tag="VT", name="VT_sb")

    def emit_k(nb, pool, tag):
        kp = pool.tile([P, QB], F32, tag=tag, name="k_ps")
        for m in range(NCC // 2):
            nc.tensor.matmul(kp, wk8[:, 2 * m:2 * m + 2, :],
                             xkv8[:, 2 * m:2 * m + 2, ts(nb, QB)],
                             start=(m == 0), stop=(m == 1), perf_mode=DR)
        nc.scalar.activation(K_sb[:, ts(nb, QB)], kp, AF.Identity,
                             bias=bk_sb, scale=1.0 / WS)

    def emit_q(nb, pool, tag):
        qp = pool.tile([P, QB], F32, tag=tag, name="q_ps")
        for m in range(NCC // 2):
            nc.tensor.matmul(qp, wq8[:, 2 * m:2 * m + 2, :],
                             xq8[:, 2 * m:2 * m + 2, ts(nb, QB)],
                             start=(m == 0), stop=(m == 1), perf_mode=DR)
        nc.scalar.activation(Q_sb[:, ts(nb, QB)], qp, AF.Identity,
                             bias=bq_sb, scale=1.0 / WS)

    def emit_vt(nt):
        vp = pst.tile([P, C], F32, tag="st", name="v_ps")
        for m in range(NCC // 2):
            nc.tensor.matmul(vp, xkv8[:, 2 * m:2 * m + 2, ts(nt, P)],
                             wv8[:, 2 * m:2 * m + 2, :],
                             start=(m == 0), stop=(m == 1), perf_mode=DR)
        nc.vector.scalar_tensor_tensor(VT_sb[:, nt, :], vp, 1.0 / WS, bv_bcast,
                                       op0=ALU.mult, op1=ALU.add)

    # K0/K1 (gated by xkv piece 0) and Q0/Q1 (xq piece 0) upfront in the
    # not-yet-rotating accumulator banks; everything else pipelines into
    # qblock 0's pair loop below.
    emit_k(0, pacc, "a0")
    emit_k(1, pacc, "a1")
    emit_q(0, pacc, "a2")
    emit_q(1, pacc, "a3")
    emit_vt(0)
    emit_vt(1)

    # ---- attention main loop ----
    def emit_st_exp(qb, t):
        """ST pair t (two row-tiled concurrent matmuls) + exp -> fp8e5 pair."""
        st0 = pst.tile([P, QB], F32, tag="st", name="st_ps")
        nc.tensor.matmul(st0, K_sb[0:DQ, ts(2 * t, P)],
                         Q_sb[0:DQ, ts(qb, QB)], start=True, stop=True)
        st1 = pst.tile([P, QB], F32, tag="st", name="st_ps")
        nc.tensor.matmul(st1, K_sb[DQ:P, ts(2 * t + 1, P)],
                         Q_sb[DQ:P, ts(qb, QB)], start=True, stop=True)
        pt = ptp.tile([P, 2, QB], F8E5, tag="pt", name="pt_sb", bufs=8)
        nc.scalar.activation(pt[:, 0, :], st0, AF.Exp, bias=negshift)
        nc.scalar.activation(pt[:, 1, :], st1, AF.Exp, bias=negshift)
        return pt

    def tail(qb, o_ps, rs_ps, xrs):
        # recip first (frees the rs bank), then the four normalization muls
        # straight from the o psum banks (each frees its bank in rotation
        # order), then residual-add + store.
        recip_b = dvp.tile([P, QB], F32, tag="recip_b", name="recip_b")
        nc.vector.reciprocal(recip_b, rs_ps)
        t1s = []
        for cc in range(NCC):
            t1 = dvp.tile([P, QB], F32, tag=f"t1_{cc}", name="t1", bufs=2)
            nc.vector.tensor_mul(t1, o_ps[cc], recip_b)
            t1s.append(t1)
        for cc in range(NCC):
            og = dvp.tile([P, QB], F32, tag="og", name="og", bufs=2)
            nc.vector.scalar_tensor_tensor(og, t1s[cc], gamma_b, xrs[cc],
                                           op0=ALU.mult, op1=ALU.add)
            nc.sync.dma_start(out_d[ts(cc, P), ts(qb, QB)], og)

    pt_next = emit_st_exp(0, 0)
    for qb in range(NQB):
        # fixed bank roles: rs reuses the bank the previous recip freed
        # (earliest release), o_cc the bank the cc-th normalization mul freed.
        o_ps = [pacc.tile([P, QB], F32, tag=f"a{cc}",
                          name=f"o_ps{cc}") for cc in range(NCC)]
        rs_ps = pacc.tile([P, QB], F32, tag="a4", name="rs_ps")
        # prefetch this qblock's residual slices (consumed by the tail)
        xrs = []
        for cc in range(NCC):
            xr = stage.tile([P, QB], F32, tag=f"xres{cc}", name="x_res", bufs=2)
            nc.sync.dma_start(xr, xq_d[ts(cc, P), ts(qb, QB)])
            xrs.append(xr)
        for t in range(NP):
            if qb == 0:
                # software-pipelined projections: VT pair one pair ahead of
                # its consumer, K block two pairs ahead, all through the st
                # psum ring.
                if t < NP - 1:
                    emit_vt(2 * t + 2)
                    emit_vt(2 * t + 3)
                if t % 2 == 1 and t < 13:
                    emit_k((t + 3) // 2, pst, "st")
            pt = pt_next
            g = qb * NP + t + 1
            if g < NQB * NP:
                pt_next = emit_st_exp(g // NP, g % NP)
            # rs first: at pair 15 its stop unblocks the recip early, and at
            # pair 0 it reuses the earliest-freed (recip) bank.
            nc.tensor.matmul(rs_ps, ones_dr, pt,
                             start=(t == 0), stop=(t == NP - 1), perf_mode=DR)
            for cc in range(NCC):
                nc.tensor.matmul(o_ps[cc], VT_sb[:, 2 * t:2 * t + 2, ts(cc, P)],
                                 pt, start=(t == 0), stop=(t == NP - 1),
                                 perf_mode=DR)
            if t == 8 and qb + 2 < NQB:
                emit_q(qb + 2, pst, "st")
        tail(qb, o_ps, rs_ps, xrs)


_NC_CACHE = {}


def _fuse_ldweights(nc):
    """Re-fuse Tile's split LDWEIGHTS+MATMUL pairs into self-loading matmuls
    so walrus's ldw-opt (background weight buffer) can overlap weight loads
    with in-flight matmuls."""
    for b in nc.m.functions[0].blocks:
        out = []
        pending = None
        for i in b.instructions:
            tn = type(i).__name__
            if tn == "InstLdweights":
                assert pending is None, "back-to-back ldweights"
                pending = i
                continue
            if tn == "InstMatmult" and pending is not None:
                i.ldweights = True
                si = pending.sync_info
                if si is not None and (si.on_wait or si.on_update):
                    if i.sync_info is None:
                        i.sync_info = mybir.SyncInfo(on_wait=[], on_update=[])
                    i.sync_info.on_wait = list(si.on_wait) + list(i.sync_info.on_wait)
                    i.sync_info.on_update = (list(si.on_update)
                                             + list(i.sync_info.on_update))
                pending = None
            out.append(i)
        assert pending is None, "trailing ldweights without matmul"
        b.instructions[:] = out


def _build():
    if "nc" in _NC_CACHE:
        return _NC_CACHE["nc"]
    nc = bacc.Bacc("TRN2", target_bir_lowering=False, debug=False, num_devices=8)
    io = {
        "xq": nc.dram_tensor("xq", [C, N], F32, kind="ExternalInput").ap(),
        "xkv": nc.dram_tensor("xkv", [C, N], F32, kind="ExternalInput").ap(),
        "wq": nc.dram_tensor("wq", [C, P], F32, kind="ExternalInput").ap(),
        "wk": nc.dram_tensor("wk", [C, P], F32, kind="ExternalInput").ap(),
        "wv": nc.dram_tensor("wv", [C, C], F32, kind="ExternalInput").ap(),
        "bq": nc.dram_tensor("bq", [P, 1], F32, kind="ExternalInput").ap(),
        "bk": nc.dram_tensor("bk", [P, 1], F32, kind="ExternalInput").ap(),
        "bv": nc.dram_tensor("bv", [1, C], F32, kind="ExternalInput").ap(),
        "gamma": nc.dram_tensor("gamma", [P, 1], F32, kind="ExternalInput").ap(),
        "out": nc.dram_tensor("out", [C, N], F32, kind="ExternalOutput").ap(),
    }
    with tile.TileContext(nc) as tc:
        _body(tc, io)
    _fuse_ldweights(nc)
    nc.compile()
    _NC_CACHE["nc"] = nc
    return nc


def make_in_maps(x1, x2, wq1, bq1, wk1, bk1, wv1, bv1,
                 wq2, bq2, wk2, bk2, wv2, bv2, gamma1, gamma2):
    """Returns the 8 per-core input dicts. Cores 0-3: out1[b]; 4-7: out2[b]."""
    f = np.ascontiguousarray
    x1f = np.asarray(x1, np.float32).reshape(B, C, N)
    x2f = np.asarray(x2, np.float32).reshape(B, C, N)

    def wdup(w):  # [DQ, C] -> [C, 2*DQ] (transposed, duplicated)
        wt = np.asarray(w, np.float32).T
        return f(np.concatenate([wt, wt], axis=1))

    def bdup(b):  # [DQ] -> [128, 1]
        bb = np.asarray(b, np.float32).reshape(DQ, 1)
        return f(np.concatenate([bb, bb], axis=0))

    maps = []
    for b in range(B):
        maps.append({
            "xq": f(x1f[b]), "xkv": f(x2f[b]),
            "wq": wdup(wq1), "wk": wdup(wk2),
            "wv": f(np.asarray(wv2, np.float32).T),
            "bq": bdup(bq1), "bk": bdup(bk2),
            "bv": f(np.asarray(bv2, np.float32).reshape(1, C)),
            "gamma": f(np.tile(np.asarray(gamma1, np.float32).reshape(1, 1), (P, 1))),
        })
    for b in range(B):
        maps.append({
            "xq": f(x2f[b]), "xkv": f(x1f[b]),
            "wq": wdup(wq2), "wk": wdup(wk1),
            "wv": f(np.asarray(wv1, np.float32).T),
            "bq": bdup(bq2), "bk": bdup(bk1),
            "bv": f(np.asarray(bv1, np.float32).reshape(1, C)),
            "gamma": f(np.tile(np.asarray(gamma2, np.float32).reshape(1, 1), (P, 1))),
        })
    return maps


def kernel(**inputs):
    nc = _build()
    in_maps = make_in_maps(**inputs)
    res = run_bass_kernel_spmd(nc, in_maps, list(range(8))).results
    out1 = np.stack([res[b]["out"].reshape(C, H, W) for b in range(B)])
    out2 = np.stack([res[B + b]["out"].reshape(C, H, W) for b in range(B)])
    return out1, out2


# revision 11
# speedup vs baseline: 1.1480x; 1.1177x over previous
"""Trainium2 Bass kernel for dual cross-attention (CotSR block).

Problem: two cross-attentions between x1, x2 [B=4, C=512, H=W=64].
  q1 = wq1@x1, k2 = wk2@x2, v2 = wv2@x2 ; att1 = softmax(q1^T k2) over keys
  out1 = x1 + gamma1 * (v2 @ att1^T)   (and symmetrically for out2)

Sharding: 8 independent (batch, direction) jobs -> one per NeuronCore.

Per-core dataflow (N = 4096 tokens, DQ = 64, C = 512), v4:
  - x arrives as fp8e4 (gpsimd DMA casts f32->fp8 in flight); xq piece 0
    first so Q0 unblocks early.
  - All projections are fp8 DoubleRow matmuls (contraction 256/pass);
    weights host-transposed, x16-scaled into fp8e4; evictions scale 1/16.
    Q/K weights host-duplicated so dq=64 lands on partitions 0-63 AND
    64-127 (enables 2x row-tiled ST matmuls).
  - K blocks and VT tiles are software-pipelined INTO qblock 0's pair loop
    (pair t only needs K/VT keytiles 2t,2t+1), so attention starts as soon
    as the first xkv DMA piece lands instead of after all projections.
  - Attention per 512-query block, over 16 key-tile PAIRS (256 keys each):
      ST:  two concurrent row-tiled bf16 matmuls (rows 0-63 / 64-127)
      PT:  exp(ST - 7) -> fp8e5 pair tile [128, 2, 512]  (global shift is
           exact for softmax: numerator and denominator both scale e^-7)
      RS:  1 DoubleRow matmul, all-ones lhsT -> BROADCAST row-sum [128,512]
      PV:  4 DoubleRow fp8 matmuls (contraction 256) accumulate [128c, 512q]
  - PSUM 5-bank accumulator rotation: qblock qb+1's first PV reuses the
    bank freed by qb's recip (earliest), later PVs the banks freed by the
    per-cc normalization muls -> near-zero qblock seam.
  PSUM: 5 banks rotating O/RS + 3 banks ST ring = 8.
"""

import numpy as np

import concourse.bass as bass
import concourse.mybir as mybir
import concourse.tile as tile
from concourse import bacc
import concourse.bass_utils as _bu

# walrus's --enable-ldw-opt=false serializes every LDWEIGHTS with its MATMUL
# (measured 379 ns/MM vs ~215 warm); enable background-weight-buffer overlap.
_orig_run_command = _bu.run_command


def _patched_run_command(argv, **kw):
    argv = ["--enable-ldw-opt=true" if a == "--enable-ldw-opt=false" else a
            for a in argv]
    return _orig_run_command(argv, **kw)


_bu.run_command = _patched_run_command
from concourse.bass_utils import run_bass_kernel_spmd
from concourse._compat import with_exitstack
from contextlib import ExitStack

F32 = mybir.dt.float32
BF16 = mybir.dt.bfloat16
F8E4 = mybir.dt.float8e4
F8E5 = mybir.dt.float8e5
AF = mybir.ActivationFunctionType
ALU = mybir.AluOpType
DR = mybir.MatmulPerfMode.DoubleRow
ts = bass.ts

B, C, H, W = 4, 512, 64, 64
N = H * W          # 4096
DQ = 64
P = 128
QB = 512           # query block (free dim of ST / moving operand)
NQB = N // QB      # 8 query blocks
NKT = N // P       # 32 key tiles
NP = NKT // 2      # 16 key-tile pairs
NCC = C // P       # 4 channel chunks
SHIFT = 7.0        # global logit shift before exp (softmax-invariant)
WS = 16.0          # fp8 weight scale (undone at psum eviction)


@with_exitstack
def _body(ctx: ExitStack, tc: "tile.TileContext", io: dict):
    nc = tc.nc
    xq_d, xkv_d, wq_d, wk_d, wv_d = io["xq"], io["xkv"], io["wq"], io["wk"], io["wv"]
    bq_d, bk_d, bv_d, gamma_d, out_d = io["bq"], io["bk"], io["bv"], io["gamma"], io["out"]

    const = ctx.enter_context(tc.tile_pool(name="const", bufs=1))
    persist = ctx.enter_context(tc.tile_pool(name="persist", bufs=1))
    wpool = ctx.enter_context(tc.tile_pool(name="wpool", bufs=1))
    stage = ctx.enter_context(tc.tile_pool(name="stage", bufs=3))
    ptp = ctx.enter_context(tc.tile_pool(name="ptp", bufs=3))
    dvp = ctx.enter_context(tc.tile_pool(name="dvp", bufs=3))
    pacc = ctx.enter_context(tc.tile_pool(name="pacc", bufs=1, space="PSUM"))
    pst = ctx.enter_context(tc.tile_pool(name="pst", bufs=3, space="PSUM"))

    # ---- constants ----
    ones_dr = const.tile([P, 2, P], F8E4, tag="ones_dr", name="ones_dr")
    nc.vector.memset(ones_dr, 1.0)
    ones_row_bf = const.tile([1, P], BF16, tag="ones_row_bf", name="ones_row_bf")
    nc.vector.memset(ones_row_bf, 1.0)
    negshift = const.tile([P, 1], F32, tag="negshift", name="negshift")
    nc.vector.memset(negshift, -SHIFT)

    # ---- small inputs (biases host-duplicated to [128,1]) ----
    bq_sb = const.tile([P, 1], F32, tag="bq", name="bq_sb")
    nc.sync.dma_start(bq_sb, bq_d)
    bk_sb = const.tile([P, 1], F32, tag="bk", name="bk_sb")
    nc.sync.dma_start(bk_sb, bk_d)
    bv_sb = const.tile([1, C], F32, tag="bv", name="bv_sb")
    nc.sync.dma_start(bv_sb, bv_d)
    bv_bf = const.tile([1, C], BF16, tag="bvbf", name="bv_bf")
    nc.vector.tensor_copy(bv_bf, bv_sb)
    gamma_b = const.tile([P, 1], F32, tag="gamma_b", name="gamma_b")
    nc.sync.dma_start(gamma_b, gamma_d)

    # bv broadcast to all partitions once: [128, C] bf16
    bvb_ps = pst.tile([P, C], F32, tag="st", name="bvb_ps")
    nc.tensor.matmul(bvb_ps, ones_row_bf, bv_bf, start=True, stop=True)
    bv_bcast = const.tile([P, C], BF16, tag="bv_bcast", name="bv_bcast")
    nc.vector.tensor_copy(bv_bcast, bvb_ps)

    # ---- weights: host pre-transposed (+ dq-duplicated for Q/K); stage f32
    # then x16-scale into fp8e4 pair-sliceable tiles ----
    wq8 = wpool.tile([P, NCC, P], F8E4, tag="wq8", name="wq8")
    wk8 = wpool.tile([P, NCC, P], F8E4, tag="wk8", name="wk8")
    for j in range(NCC):
        for (src_d, dst) in ((wq_d, wq8), (wk_d, wk8)):
            wst = stage.tile([P, P], F32, tag="w_stage", name="w_st")
            nc.sync.dma_start(wst, src_d[ts(j, P), :])
            nc.vector.tensor_scalar_mul(dst[:, j, :], wst, WS)
    wv8 = wpool.tile([P, NCC, C], F8E4, tag="wv8", name="wv8")
    for j in range(NCC):
        wst2 = stage.tile([P, C], F32, tag="w_stage2", name="w_st2")
        nc.sync.dma_start(wst2, wv_d[ts(j, P), :])
        nc.vector.tensor_scalar_mul(wv8[:, j, :], wst2, WS)

    # ---- x resident fp8e4; gpsimd DMA casts f32->fp8 in flight.
    # xq piece 0 first (unblocks Q0), then all xkv, then the rest of xq.
    xq8 = persist.tile([P, NCC, N], F8E4, tag="xq8", name="xq8")
    xkv8 = persist.tile([P, NCC, N], F8E4, tag="xkv8", name="xkv8")
    for cc in range(NCC):
        nc.gpsimd.dma_start(xq8[:, cc, ts(0, 1024)], xq_d[ts(cc, P), ts(0, 1024)])
    for h in range(4):  # 1024-col pieces
        for cc in range(NCC):
            nc.gpsimd.dma_start(xkv8[:, cc, ts(h, 1024)],
                                xkv_d[ts(cc, P), ts(h, 1024)])
    for h in range(1, 4):
        for cc in range(NCC):
            nc.gpsimd.dma_start(xq8[:, cc, ts(h, 1024)],
                                xq_d[ts(cc, P), ts(h, 1024)])

    # ---- projection helpers (fp8 DoubleRow, contraction 2x128/pass) ----
    # Q/K land duplicated on both partition halves: [128(2x dq), N]
    Q_sb = persist.tile([P, N], BF16, tag="Q", name="Q_sb")
    K_sb = persist.tile([P, N], BF16, tag="K", name="K_sb")
    VT_sb = persist.tile([P, NKT, C], F8E4, tag="VT", name="VT_sb")

    def emit_k(nb, pool, tag):
        kp = pool.tile([P, QB], F32, tag=tag, name="k_ps")
        for m in range(NCC // 2):
            nc.tensor.matmul(kp, wk8[:, 2 * m:2 * m + 2, :],
                             xkv8[:, 2 * m:2 * m + 2, ts(nb, QB)],
                             start=(m == 0), stop=(m == 1), perf_mode=DR)
        nc.scalar.activation(K_sb[:, ts(nb, QB)], kp, AF.Identity,
                             bias=bk_sb, scale=1.0 / WS)

    def emit_q(nb, pool, tag):
        qp = pool.tile([P, QB], F32, tag=tag, name="q_ps")
        for m in range(NCC // 2):
            nc.tensor.matmul(qp, wq8[:, 2 * m:2 * m + 2, :],
                             xq8[:, 2 * m:2 * m + 2, ts(nb, QB)],
                             start=(m == 0), stop=(m == 1), perf_mode=DR)
        nc.scalar.activation(Q_sb[:, ts(nb, QB)], qp, AF.Identity,
                             bias=bq_sb, scale=1.0 / WS)

    def emit_vt(nt):
        vp = pst.tile([P, C], F32, tag="st", name="v_ps")
        for m in range(NCC // 2):
            nc.tensor.matmul(vp, xkv8[:, 2 * m:2 * m + 2, ts(nt, P)],
                             wv8[:, 2 * m:2 * m + 2, :],
                             start=(m == 0), stop=(m == 1), perf_mode=DR)
        nc.vector.scalar_tensor_tensor(VT_sb[:, nt, :], vp, 1.0 / WS, bv_bcast,
                                       op0=ALU.mult, op1=ALU.add)

    # K0/K1 (gated by xkv piece 0) and Q0/Q1 (xq piece 0) upfront in the
    # not-yet-rotating accumulator banks; everything else pipelines into
    # qblock 0's pair loop below.
    emit_k(0, pacc, "a0")
    emit_k(1, pacc, "a1")
    emit_q(0, pacc, "a2")
    emit_q(1, pacc, "a3")
    emit_vt(0)
    emit_vt(1)

    # ---- attention main loop ----
    def emit_st_exp(qb, t):
        """ST pair t (two row-tiled concurrent matmuls) + exp -> fp8e5 pair."""
        st0 = pst.tile([P, QB], F32, tag="st", name="st_ps")
        nc.tensor.matmul(st0, K_sb[0:DQ, ts(2 * t, P)],
                         Q_sb[0:DQ, ts(qb, QB)], start=True, stop=True)
        st1 = pst.tile([P, QB], F32, tag="st", name="st_ps")
        nc.tensor.matmul(st1, K_sb[DQ:P, ts(2 * t + 1, P)],
                         Q_sb[DQ:P, ts(qb, QB)], start=True, stop=True)
        pt = ptp.tile([P, 2, QB], F8E5, tag="pt", name="pt_sb", bufs=8)
        nc.scalar.activation(pt[:, 0, :], st0, AF.Exp, bias=negshift)
        nc.scalar.activation(pt[:, 1, :], st1, AF.Exp, bias=negshift)
        return pt

    def tail(qb, o_ps, rs_ps, xrs):
        # recip first (frees the rs bank), then the four normalization muls
        # straight from the o psum banks (each frees its bank in rotation
        # order), then residual-add + store.
        recip_b = dvp.tile([P, QB], F32, tag="recip_b", name="recip_b")
        nc.vector.reciprocal_approx_fast(recip_b, rs_ps)
        t1s = []
        for cc in range(NCC):
            t1 = dvp.tile([P, QB], F32, tag=f"t1_{cc}", name="t1", bufs=2)
            nc.vector.tensor_mul(t1, o_ps[cc], recip_b)
            t1s.append(t1)
        for cc in range(NCC):
            og = dvp.tile([P, QB], F32, tag="og", name="og", bufs=2)
            nc.vector.scalar_tensor_tensor(og, t1s[cc], gamma_b, xrs[cc],
                                           op0=ALU.mult, op1=ALU.add)
            nc.sync.dma_start(out_d[ts(cc, P), ts(qb, QB)], og)

    pt_next = emit_st_exp(0, 0)
    for qb in range(NQB):
        # fixed bank roles: rs reuses the bank the previous recip freed
        # (earliest release), o_cc the bank the cc-th normalization mul freed.
        o_ps = [pacc.tile([P, QB], F32, tag=f"a{cc}",
                          name=f"o_ps{cc}") for cc in range(NCC)]
        rs_ps = pacc.tile([P, QB], F32, tag="a4", name="rs_ps")
        # prefetch this qblock's residual slices (consumed by the tail)
        xrs = []
        for cc in range(NCC):
            xr = stage.tile([P, QB], F32, tag=f"xres{cc}", name="x_res", bufs=2)
            nc.sync.dma_start(xr, xq_d[ts(cc, P), ts(qb, QB)])
            xrs.append(xr)
        for t in range(NP):
            if qb == 0:
                # software-pipelined projections: VT pair one pair ahead of
                # its consumer, K block two pairs ahead, all through the st
                # psum ring.
                if t < NP - 1:
                    emit_vt(2 * t + 2)
                    emit_vt(2 * t + 3)
                if t % 2 == 1 and t < 13:
                    emit_k((t + 3) // 2, pst, "st")
            pt = pt_next
            g = qb * NP + t + 1
            if g < NQB * NP:
                pt_next = emit_st_exp(g // NP, g % NP)
            # rs first: at pair 15 its stop unblocks the recip early, and at
            # pair 0 it reuses the earliest-freed (recip) bank.
            nc.tensor.matmul(rs_ps, ones_dr, pt,
                             start=(t == 0), stop=(t == NP - 1), perf_mode=DR)
            for cc in range(NCC):
                nc.tensor.matmul(o_ps[cc], VT_sb[:, 2 * t:2 * t + 2, ts(cc, P)],
                                 pt, start=(t == 0), stop=(t == NP - 1),
                                 perf_mode=DR)
            if t == 8 and qb + 2 < NQB:
                emit_q(qb + 2, pst, "st")
        tail(qb, o_ps, rs_ps, xrs)


_NC_CACHE = {}


def _fuse_ldweights(nc):
    """Re-fuse Tile's split LDWEIGHTS+MATMUL pairs into self-loading matmuls
    so walrus's ldw-opt (background weight buffer) can overlap weight loads
    with in-flight matmuls."""
    for b in nc.m.functions[0].blocks:
        out = []
        pending = None
        for i in b.instructions:
            tn = type(i).__name__
            if tn == "InstLdweights":
                assert pending is None, "back-to-back ldweights"
                pending = i
                continue
            if tn == "InstMatmult" and pending is not None:
                i.ldweights = True
                si = pending.sync_info
                if si is not None and (si.on_wait or si.on_update):
                    if i.sync_info is None:
                        i.sync_info = mybir.SyncInfo(on_wait=[], on_update=[])
                    i.sync_info.on_wait = list(si.on_wait) + list(i.sync_info.on_wait)
                    i.sync_info.on_update = (list(si.on_update)
                                             + list(i.sync_info.on_update))
                pending = None
            out.append(i)
        assert pending is None, "trailing ldweights without matmul"
        b.instructions[:] = out


def _build():
    if "nc" in _NC_CACHE:
        return _NC_CACHE["nc"]
    nc = bacc.Bacc("TRN2", target_bir_lowering=False, debug=False, num_devices=8)
    io = {
        "xq": nc.dram_tensor("xq", [C, N], F32, kind="ExternalInput").ap(),
        "xkv": nc.dram_tensor("xkv", [C, N], F32, kind="ExternalInput").ap(),
        "wq": nc.dram_tensor("wq", [C, P], F32, kind="ExternalInput").ap(),
        "wk": nc.dram_tensor("wk", [C, P], F32, kind="ExternalInput").ap(),
        "wv": nc.dram_tensor("wv", [C, C], F32, kind="ExternalInput").ap(),
        "bq": nc.dram_tensor("bq", [P, 1], F32, kind="ExternalInput").ap(),
        "bk": nc.dram_tensor("bk", [P, 1], F32, kind="ExternalInput").ap(),
        "bv": nc.dram_tensor("bv", [1, C], F32, kind="ExternalInput").ap(),
        "gamma": nc.dram_tensor("gamma", [P, 1], F32, kind="ExternalInput").ap(),
        "out": nc.dram_tensor("out", [C, N], F32, kind="ExternalOutput").ap(),
    }
    with tile.TileContext(nc) as tc:
        _body(tc, io)
    _fuse_ldweights(nc)
    nc.compile()
    _NC_CACHE["nc"] = nc
    return nc


def make_in_maps(x1, x2, wq1, bq1, wk1, bk1, wv1, bv1,
                 wq2, bq2, wk2, bk2, wv2, bv2, gamma1, gamma2):
    """Returns the 8 per-core input dicts. Cores 0-3: out1[b]; 4-7: out2[b]."""
    f = np.ascontiguousarray
    x1f = np.asarray(x1, np.float32).reshape(B, C, N)
    x2f = np.asarray(x2, np.float32).reshape(B, C, N)

    def wdup(w):  # [DQ, C] -> [C, 2*DQ] (transposed, duplicated)
        wt = np.asarray(w, np.float32).T
        return f(np.concatenate([wt, wt], axis=1))

    def bdup(b):  # [DQ] -> [128, 1]
        bb = np.asarray(b, np.float32).reshape(DQ, 1)
        return f(np.concatenate([bb, bb], axis=0))

    maps = []
    for b in range(B):
        maps.append({
            "xq": f(x1f[b]), "xkv": f(x2f[b]),
            "wq": wdup(wq1), "wk": wdup(wk2),
            "wv": f(np.asarray(wv2, np.float32).T),
            "bq": bdup(bq1), "bk": bdup(bk2),
            "bv": f(np.asarray(bv2, np.float32).reshape(1, C)),
            "gamma": f(np.tile(np.asarray(gamma1, np.float32).reshape(1, 1), (P, 1))),
        })
    for b in range(B):
        maps.append({
            "xq": f(x2f[b]), "xkv": f(x1f[b]),
            "wq": wdup(wq2), "wk": wdup(wk1),
            "wv": f(np.asarray(wv1, np.float32).T),
            "bq": bdup(bq2), "bk": bdup(bk1),
            "bv": f(np.asarray(bv1, np.float32).reshape(1, C)),
            "gamma": f(np.tile(np.asarray(gamma2, np.float32).reshape(1, 1), (P, 1))),
        })
    return maps


def kernel(**inputs):
    nc = _build()
    in_maps = make_in_maps(**inputs)
    res = run_bass_kernel_spmd(nc, in_maps, list(range(8))).results
    out1 = np.stack([res[b]["out"].reshape(C, H, W) for b in range(B)])
    out2 = np.stack([res[B + b]["out"].reshape(C, H, W) for b in range(B)])
    return out1, out2


# revision 14
# speedup vs baseline: 1.1590x; 1.0096x over previous
"""Trainium2 Bass kernel for dual cross-attention (CotSR block).

Problem: two cross-attentions between x1, x2 [B=4, C=512, H=W=64].
  q1 = wq1@x1, k2 = wk2@x2, v2 = wv2@x2 ; att1 = softmax(q1^T k2) over keys
  out1 = x1 + gamma1 * (v2 @ att1^T)   (and symmetrically for out2)

Sharding: 8 independent (batch, direction) jobs -> one per NeuronCore.

Per-core dataflow (N = 4096 tokens, DQ = 64, C = 512), v4:
  - x arrives as fp8e4 (gpsimd DMA casts f32->fp8 in flight); xq piece 0
    first so Q0 unblocks early.
  - All projections are fp8 DoubleRow matmuls (contraction 256/pass);
    weights host-transposed, x16-scaled into fp8e4; evictions scale 1/16.
    Q/K weights host-duplicated so dq=64 lands on partitions 0-63 AND
    64-127 (enables 2x row-tiled ST matmuls).
  - K blocks and VT tiles are software-pipelined INTO qblock 0's pair loop
    (pair t only needs K/VT keytiles 2t,2t+1), so attention starts as soon
    as the first xkv DMA piece lands instead of after all projections.
  - Attention per 512-query block, over 16 key-tile PAIRS (256 keys each):
      ST:  two concurrent row-tiled bf16 matmuls (rows 0-63 / 64-127)
      PT:  exp(ST - 7) -> fp8e5 pair tile [128, 2, 512]  (global shift is
           exact for softmax: numerator and denominator both scale e^-7)
      RS:  1 DoubleRow matmul, all-ones lhsT -> BROADCAST row-sum [128,512]
      PV:  4 DoubleRow fp8 matmuls (contraction 256) accumulate [128c, 512q]
  - PSUM 5-bank accumulator rotation: qblock qb+1's first PV reuses the
    bank freed by qb's recip (earliest), later PVs the banks freed by the
    per-cc normalization muls -> near-zero qblock seam.
  PSUM: 5 banks rotating O/RS + 3 banks ST ring = 8.
"""

import numpy as np

import concourse.bass as bass
import concourse.mybir as mybir
import concourse.tile as tile
from concourse import bacc
import concourse.bass_utils as _bu

# walrus's --enable-ldw-opt=false serializes every LDWEIGHTS with its MATMUL
# (measured 379 ns/MM vs ~215 warm); enable background-weight-buffer overlap.
_orig_run_command = _bu.run_command


def _patched_run_command(argv, **kw):
    argv = ["--enable-ldw-opt=true" if a == "--enable-ldw-opt=false" else a
            for a in argv]
    return _orig_run_command(argv, **kw)


_bu.run_command = _patched_run_command
from concourse.bass_utils import run_bass_kernel_spmd
from concourse._compat import with_exitstack
from contextlib import ExitStack

F32 = mybir.dt.float32
BF16 = mybir.dt.bfloat16
F8E4 = mybir.dt.float8e4
F8E5 = mybir.dt.float8e5
AF = mybir.ActivationFunctionType
ALU = mybir.AluOpType
DR = mybir.MatmulPerfMode.DoubleRow
ts = bass.ts

B, C, H, W = 4, 512, 64, 64
N = H * W          # 4096
DQ = 64
P = 128
QB = 512           # query block (free dim of ST / moving operand)
NQB = N // QB      # 8 query blocks
NKT = N // P       # 32 key tiles
NP = NKT // 2      # 16 key-tile pairs
NCC = C // P       # 4 channel chunks
SHIFT = 7.0        # global logit shift before exp (softmax-invariant)
WS = 16.0          # fp8 weight scale (undone at psum eviction)


@with_exitstack
def _body(ctx: ExitStack, tc: "tile.TileContext", io: dict):
    nc = tc.nc
    xq_d, xkv_d, wq_d, wk_d, wv_d = io["xq"], io["xkv"], io["wq"], io["wk"], io["wv"]
    bq_d, bk_d, bv_d, gamma_d, out_d = io["bq"], io["bk"], io["bv"], io["gamma"], io["out"]

    const = ctx.enter_context(tc.tile_pool(name="const", bufs=1))
    persist = ctx.enter_context(tc.tile_pool(name="persist", bufs=1))
    wpool = ctx.enter_context(tc.tile_pool(name="wpool", bufs=1))
    stage = ctx.enter_context(tc.tile_pool(name="stage", bufs=3))
    ptp = ctx.enter_context(tc.tile_pool(name="ptp", bufs=3))
    dvp = ctx.enter_context(tc.tile_pool(name="dvp", bufs=3))
    pacc = ctx.enter_context(tc.tile_pool(name="pacc", bufs=1, space="PSUM"))
    pst = ctx.enter_context(tc.tile_pool(name="pst", bufs=3, space="PSUM"))

    # ---- constants ----
    ones_dr = const.tile([P, 2, P], F8E4, tag="ones_dr", name="ones_dr")
    nc.vector.memset(ones_dr, 1.0)
    ones_row_bf = const.tile([1, P], BF16, tag="ones_row_bf", name="ones_row_bf")
    nc.vector.memset(ones_row_bf, 1.0)
    negshift = const.tile([P, 1], F32, tag="negshift", name="negshift")
    nc.vector.memset(negshift, -SHIFT)

    # ---- small inputs (biases host-duplicated to [128,1]) ----
    bq_sb = const.tile([P, 1], F32, tag="bq", name="bq_sb")
    nc.sync.dma_start(bq_sb, bq_d)
    bk_sb = const.tile([P, 1], F32, tag="bk", name="bk_sb")
    nc.sync.dma_start(bk_sb, bk_d)
    bv_sb = const.tile([1, C], F32, tag="bv", name="bv_sb")
    nc.sync.dma_start(bv_sb, bv_d)
    bv_bf = const.tile([1, C], BF16, tag="bvbf", name="bv_bf")
    nc.vector.tensor_copy(bv_bf, bv_sb)
    gamma_b = const.tile([P, 1], F32, tag="gamma_b", name="gamma_b")
    nc.sync.dma_start(gamma_b, gamma_d)

    # bv broadcast to all partitions once: [128, C] bf16
    bvb_ps = pst.tile([P, C], F32, tag="st", name="bvb_ps")
    nc.tensor.matmul(bvb_ps, ones_row_bf, bv_bf, start=True, stop=True)
    bv_bcast = const.tile([P, C], BF16, tag="bv_bcast", name="bv_bcast")
    nc.vector.tensor_copy(bv_bcast, bvb_ps)

    # ---- weights: host pre-transposed (+ dq-duplicated for Q/K); stage f32
    # then x16-scale into fp8e4 pair-sliceable tiles ----
    wq8 = wpool.tile([P, NCC, P], F8E4, tag="wq8", name="wq8")
    wk8 = wpool.tile([P, NCC, P], F8E4, tag="wk8", name="wk8")
    for j in range(NCC):
        for (src_d, dst) in ((wq_d, wq8), (wk_d, wk8)):
            wst = stage.tile([P, P], F32, tag="w_stage", name="w_st")
            nc.sync.dma_start(wst, src_d[ts(j, P), :])
            nc.vector.tensor_scalar_mul(dst[:, j, :], wst, WS)
    wv8 = wpool.tile([P, NCC, C], F8E4, tag="wv8", name="wv8")
    for j in range(NCC):
        wst2 = stage.tile([P, C], F32, tag="w_stage2", name="w_st2")
        nc.sync.dma_start(wst2, wv_d[ts(j, P), :])
        nc.vector.tensor_scalar_mul(wv8[:, j, :], wst2, WS)

    # ---- x resident fp8e4; gpsimd DMA casts f32->fp8 in flight.
    # xq piece 0 first (unblocks Q0), then all xkv, then the rest of xq.
    xq8 = persist.tile([P, NCC, N], F8E4, tag="xq8", name="xq8")
    xkv8 = persist.tile([P, NCC, N], F8E4, tag="xkv8", name="xkv8")
    for cc in range(NCC):
        nc.gpsimd.dma_start(xq8[:, cc, ts(0, 1024)], xq_d[ts(cc, P), ts(0, 1024)])
    # piece 0 of xkv in 512-col sub-chunks: K0 (cols 0-511) unlocks sooner
    for s in range(2):
        for cc in range(NCC):
            nc.gpsimd.dma_start(xkv8[:, cc, ts(s, 512)],
                                xkv_d[ts(cc, P), ts(s, 512)])
    for h in range(1, 4):  # remaining 1024-col pieces
        for cc in range(NCC):
            nc.gpsimd.dma_start(xkv8[:, cc, ts(h, 1024)],
                                xkv_d[ts(cc, P), ts(h, 1024)])
    for h in range(1, 4):
        for cc in range(NCC):
            nc.gpsimd.dma_start(xq8[:, cc, ts(h, 1024)],
                                xq_d[ts(cc, P), ts(h, 1024)])

    # ---- projection helpers (fp8 DoubleRow, contraction 2x128/pass) ----
    # Q/K land duplicated on both partition halves: [128(2x dq), N]
    Q_sb = persist.tile([P, N], BF16, tag="Q", name="Q_sb")
    K_sb = persist.tile([P, N], BF16, tag="K", name="K_sb")
    VT_sb = persist.tile([P, NKT, C], F8E4, tag="VT", name="VT_sb")

    def emit_k(nb, pool, tag):
        kp = pool.tile([P, QB], F32, tag=tag, name="k_ps")
        for m in range(NCC // 2):
            nc.tensor.matmul(kp, wk8[:, 2 * m:2 * m + 2, :],
                             xkv8[:, 2 * m:2 * m + 2, ts(nb, QB)],
                             start=(m == 0), stop=(m == 1), perf_mode=DR)
        nc.scalar.activation(K_sb[:, ts(nb, QB)], kp, AF.Identity,
                             bias=bk_sb, scale=1.0 / WS)

    def emit_q(nb, pool, tag):
        qp = pool.tile([P, QB], F32, tag=tag, name="q_ps")
        for m in range(NCC // 2):
            nc.tensor.matmul(qp, wq8[:, 2 * m:2 * m + 2, :],
                             xq8[:, 2 * m:2 * m + 2, ts(nb, QB)],
                             start=(m == 0), stop=(m == 1), perf_mode=DR)
        nc.scalar.activation(Q_sb[:, ts(nb, QB)], qp, AF.Identity,
                             bias=bq_sb, scale=1.0 / WS)

    def emit_vt(nt):
        vp = pst.tile([P, C], F32, tag="st", name="v_ps")
        for m in range(NCC // 2):
            nc.tensor.matmul(vp, xkv8[:, 2 * m:2 * m + 2, ts(nt, P)],
                             wv8[:, 2 * m:2 * m + 2, :],
                             start=(m == 0), stop=(m == 1), perf_mode=DR)
        nc.vector.scalar_tensor_tensor(VT_sb[:, nt, :], vp, 1.0 / WS, bv_bcast,
                                       op0=ALU.mult, op1=ALU.add)

    # K0/K1 (gated by xkv piece 0) and Q0/Q1 (xq piece 0) upfront in the
    # not-yet-rotating accumulator banks; everything else pipelines into
    # qblock 0's pair loop below.
    emit_k(0, pacc, "a0")
    emit_k(1, pacc, "a1")
    emit_q(0, pacc, "a2")
    emit_q(1, pacc, "a3")
    emit_vt(0)
    emit_vt(1)

    # ---- attention main loop ----
    def emit_st_exp(qb, t):
        """ST pair t (two row-tiled concurrent matmuls) + exp -> fp8e5 pair."""
        st0 = pst.tile([P, QB], F32, tag="st", name="st_ps")
        nc.tensor.matmul(st0, K_sb[0:DQ, ts(2 * t, P)],
                         Q_sb[0:DQ, ts(qb, QB)], start=True, stop=True)
        st1 = pst.tile([P, QB], F32, tag="st", name="st_ps")
        nc.tensor.matmul(st1, K_sb[DQ:P, ts(2 * t + 1, P)],
                         Q_sb[DQ:P, ts(qb, QB)], start=True, stop=True)
        pt = ptp.tile([P, 2, QB], F8E5, tag="pt", name="pt_sb", bufs=8)
        nc.scalar.activation(pt[:, 0, :], st0, AF.Exp, bias=negshift)
        nc.scalar.activation(pt[:, 1, :], st1, AF.Exp, bias=negshift)
        return pt

    def tail(qb, o_ps, rs_ps, xrs, last=False):
        # recip first (frees the rs bank). Mid-kernel: all four bank-freeing
        # muls before the residual stts (next qblock's PVs want the banks).
        # Last qblock: interleave per-cc so the final stores start earliest.
        recip_b = dvp.tile([P, QB], F32, tag="recip_b", name="recip_b")
        nc.vector.reciprocal_approx_fast(recip_b, rs_ps)
        t1s = []
        for cc in range(NCC):
            t1 = dvp.tile([P, QB], F32, tag=f"t1_{cc}", name="t1", bufs=2)
            nc.vector.tensor_mul(t1, o_ps[cc], recip_b)
            t1s.append(t1)
            if last:
                og = dvp.tile([P, QB], F32, tag="og", name="og", bufs=2)
                nc.vector.scalar_tensor_tensor(og, t1, gamma_b, xrs[cc],
                                               op0=ALU.mult, op1=ALU.add)
                nc.sync.dma_start(out_d[ts(cc, P), ts(qb, QB)], og)
        if not last:
            for cc in range(NCC):
                og = dvp.tile([P, QB], F32, tag="og", name="og", bufs=2)
                nc.vector.scalar_tensor_tensor(og, t1s[cc], gamma_b, xrs[cc],
                                               op0=ALU.mult, op1=ALU.add)
                nc.sync.dma_start(out_d[ts(cc, P), ts(qb, QB)], og)

    pt_next = emit_st_exp(0, 0)
    for qb in range(NQB):
        # fixed bank roles: rs reuses the bank the previous recip freed
        # (earliest release), o_cc the bank the cc-th normalization mul freed.
        o_ps = [pacc.tile([P, QB], F32, tag=f"a{cc}",
                          name=f"o_ps{cc}") for cc in range(NCC)]
        rs_ps = pacc.tile([P, QB], F32, tag="a4", name="rs_ps")
        # prefetch this qblock's residual slices (consumed by the tail)
        xrs = []
        for cc in range(NCC):
            xr = stage.tile([P, QB], F32, tag=f"xres{cc}", name="x_res", bufs=2)
            nc.sync.dma_start(xr, xq_d[ts(cc, P), ts(qb, QB)])
            xrs.append(xr)
        for t in range(NP):
            if qb == 0:
                # software-pipelined projections: VT pair one pair ahead of
                # its consumer, K block two pairs ahead, all through the st
                # psum ring.
                if t < NP - 1:
                    emit_vt(2 * t + 2)
                    emit_vt(2 * t + 3)
                if t % 2 == 1 and t < 13:
                    emit_k((t + 3) // 2, pst, "st")
            pt = pt_next
            g = qb * NP + t + 1
            if g < NQB * NP:
                pt_next = emit_st_exp(g // NP, g % NP)
            # rs first: at pair 15 its stop unblocks the recip early, and at
            # pair 0 it reuses the earliest-freed (recip) bank.
            nc.tensor.matmul(rs_ps, ones_dr, pt,
                             start=(t == 0), stop=(t == NP - 1), perf_mode=DR)
            for cc in range(NCC):
                nc.tensor.matmul(o_ps[cc], VT_sb[:, 2 * t:2 * t + 2, ts(cc, P)],
                                 pt, start=(t == 0), stop=(t == NP - 1),
                                 perf_mode=DR)
            if t == 8 and qb + 2 < NQB:
                emit_q(qb + 2, pst, "st")
        tail(qb, o_ps, rs_ps, xrs, last=(qb == NQB - 1))


_NC_CACHE = {}


def _fuse_ldweights(nc):
    """Re-fuse Tile's split LDWEIGHTS+MATMUL pairs into self-loading matmuls
    so walrus's ldw-opt (background weight buffer) can overlap weight loads
    with in-flight matmuls."""
    for b in nc.m.functions[0].blocks:
        out = []
        pending = None
        for i in b.instructions:
            tn = type(i).__name__
            if tn == "InstLdweights":
                assert pending is None, "back-to-back ldweights"
                pending = i
                continue
            if tn == "InstMatmult" and pending is not None:
                i.ldweights = True
                si = pending.sync_info
                if si is not None and (si.on_wait or si.on_update):
                    if i.sync_info is None:
                        i.sync_info = mybir.SyncInfo(on_wait=[], on_update=[])
                    i.sync_info.on_wait = list(si.on_wait) + list(i.sync_info.on_wait)
                    i.sync_info.on_update = (list(si.on_update)
                                             + list(i.sync_info.on_update))
                pending = None
            out.append(i)
        assert pending is None, "trailing ldweights without matmul"
        b.instructions[:] = out


def _build():
    if "nc" in _NC_CACHE:
        return _NC_CACHE["nc"]
    nc = bacc.Bacc("TRN2", target_bir_lowering=False, debug=False, num_devices=8)
    io = {
        "xq": nc.dram_tensor("xq", [C, N], F32, kind="ExternalInput").ap(),
        "xkv": nc.dram_tensor("xkv", [C, N], F32, kind="ExternalInput").ap(),
        "wq": nc.dram_tensor("wq", [C, P], F32, kind="ExternalInput").ap(),
        "wk": nc.dram_tensor("wk", [C, P], F32, kind="ExternalInput").ap(),
        "wv": nc.dram_tensor("wv", [C, C], F32, kind="ExternalInput").ap(),
        "bq": nc.dram_tensor("bq", [P, 1], F32, kind="ExternalInput").ap(),
        "bk": nc.dram_tensor("bk", [P, 1], F32, kind="ExternalInput").ap(),
        "bv": nc.dram_tensor("bv", [1, C], F32, kind="ExternalInput").ap(),
        "gamma": nc.dram_tensor("gamma", [P, 1], F32, kind="ExternalInput").ap(),
        "out": nc.dram_tensor("out", [C, N], F32, kind="ExternalOutput").ap(),
    }
    with tile.TileContext(nc) as tc:
        _body(tc, io)
    _fuse_ldweights(nc)
    nc.compile()
    _NC_CACHE["nc"] = nc
    return nc


def make_in_maps(x1, x2, wq1, bq1, wk1, bk1, wv1, bv1,
                 wq2, bq2, wk2, bk2, wv2, bv2, gamma1, gamma2):
    """Returns the 8 per-core input dicts. Cores 0-3: out1[b]; 4-7: out2[b]."""
    f = np.ascontiguousarray
    x1f = np.asarray(x1, np.float32).reshape(B, C, N)
    x2f = np.asarray(x2, np.float32).reshape(B, C, N)

    def wdup(w):  # [DQ, C] -> [C, 2*DQ] (transposed, duplicated)
        wt = np.asarray(w, np.float32).T
        return f(np.concatenate([wt, wt], axis=1))

    def bdup(b):  # [DQ] -> [128, 1]
        bb = np.asarray(b, np.float32).reshape(DQ, 1)
        return f(np.concatenate([bb, bb], axis=0))

    maps = []
    for b in range(B):
        maps.append({
            "xq": f(x1f[b]), "xkv": f(x2f[b]),
            "wq": wdup(wq1), "wk": wdup(wk2),
            "wv": f(np.asarray(wv2, np.float32).T),
            "bq": bdup(bq1), "bk": bdup(bk2),
            "bv": f(np.asarray(bv2, np.float32).reshape(1, C)),
            "gamma": f(np.tile(np.asarray(gamma1, np.float32).reshape(1, 1), (P, 1))),
        })
    for b in range(B):
        maps.append({
            "xq": f(x2f[b]), "xkv": f(x1f[b]),
            "wq": wdup(wq2), "wk": wdup(wk1),
            "wv": f(np.asarray(wv1, np.float32).T),
            "bq": bdup(bq2), "bk": bdup(bk1),
            "bv": f(np.asarray(bv1, np.float32).reshape(1, C)),
            "gamma": f(np.tile(np.asarray(gamma2, np.float32).reshape(1, 1), (P, 1))),
        })
    return maps


def kernel(**inputs):
    nc = _build()
    in_maps = make_in_maps(**inputs)
    res = run_bass_kernel_spmd(nc, in_maps, list(range(8))).results
    out1 = np.stack([res[b]["out"].reshape(C, H, W) for b in range(B)])
    out2 = np.stack([res[B + b]["out"].reshape(C, H, W) for b in range(B)])
    return out1, out2


# revision 15
# speedup vs baseline: 1.1959x; 1.0318x over previous
"""Trainium2 Bass kernel for dual cross-attention (CotSR block).

Problem: two cross-attentions between x1, x2 [B=4, C=512, H=W=64].
  q1 = wq1@x1, k2 = wk2@x2, v2 = wv2@x2 ; att1 = softmax(q1^T k2) over keys
  out1 = x1 + gamma1 * (v2 @ att1^T)   (and symmetrically for out2)

Sharding: 8 independent (batch, direction) jobs -> one per NeuronCore.

Per-core dataflow (N = 4096 tokens, DQ = 64, C = 512), v4:
  - x arrives as fp8e4 (gpsimd DMA casts f32->fp8 in flight); xq piece 0
    first so Q0 unblocks early.
  - All projections are fp8 DoubleRow matmuls (contraction 256/pass);
    weights host-transposed, x16-scaled into fp8e4; evictions scale 1/16.
    Q/K weights host-duplicated so dq=64 lands on partitions 0-63 AND
    64-127 (enables 2x row-tiled ST matmuls).
  - K blocks and VT tiles are software-pipelined INTO qblock 0's pair loop
    (pair t only needs K/VT keytiles 2t,2t+1), so attention starts as soon
    as the first xkv DMA piece lands instead of after all projections.
  - Attention per 512-query block, over 16 key-tile PAIRS (256 keys each):
      ST:  two concurrent row-tiled bf16 matmuls (rows 0-63 / 64-127)
      PT:  exp(ST - 7) -> fp8e5 pair tile [128, 2, 512]  (global shift is
           exact for softmax: numerator and denominator both scale e^-7)
      RS:  1 DoubleRow matmul, all-ones lhsT -> BROADCAST row-sum [128,512]
      PV:  4 DoubleRow fp8 matmuls (contraction 256) accumulate [128c, 512q]
  - PSUM 5-bank accumulator rotation: qblock qb+1's first PV reuses the
    bank freed by qb's recip (earliest), later PVs the banks freed by the
    per-cc normalization muls -> near-zero qblock seam.
  PSUM: 5 banks rotating O/RS + 3 banks ST ring = 8.
"""

import numpy as np

import concourse.bass as bass
import concourse.mybir as mybir
import concourse.tile as tile
from concourse import bacc
import concourse.bass_utils as _bu

# walrus's --enable-ldw-opt=false serializes every LDWEIGHTS with its MATMUL
# (measured 379 ns/MM vs ~215 warm); enable background-weight-buffer overlap.
_orig_run_command = _bu.run_command


def _patched_run_command(argv, **kw):
    argv = ["--enable-ldw-opt=true" if a == "--enable-ldw-opt=false" else a
            for a in argv]
    return _orig_run_command(argv, **kw)


_bu.run_command = _patched_run_command
from concourse.bass_utils import run_bass_kernel_spmd
from concourse._compat import with_exitstack
from contextlib import ExitStack

F32 = mybir.dt.float32
BF16 = mybir.dt.bfloat16
F8E4 = mybir.dt.float8e4
F8E5 = mybir.dt.float8e5
AF = mybir.ActivationFunctionType
ALU = mybir.AluOpType
DR = mybir.MatmulPerfMode.DoubleRow
ts = bass.ts

B, C, H, W = 4, 512, 64, 64
N = H * W          # 4096
DQ = 64
P = 128
QB = 512           # query block (free dim of ST / moving operand)
NQB = N // QB      # 8 query blocks
NKT = N // P       # 32 key tiles
NP = NKT // 2      # 16 key-tile pairs
NCC = C // P       # 4 channel chunks
SHIFT = 7.0        # global logit shift before exp (softmax-invariant)
WS = 16.0          # fp8 weight scale (undone at psum eviction)


@with_exitstack
def _body(ctx: ExitStack, tc: "tile.TileContext", io: dict):
    nc = tc.nc
    xq_d, xkv_d, wq_d, wk_d, wv_d = io["xq"], io["xkv"], io["wq"], io["wk"], io["wv"]
    bq_d, bk_d, bv_d, gamma_d, out_d = io["bq"], io["bk"], io["bv"], io["gamma"], io["out"]

    const = ctx.enter_context(tc.tile_pool(name="const", bufs=1))
    persist = ctx.enter_context(tc.tile_pool(name="persist", bufs=1))
    wpool = ctx.enter_context(tc.tile_pool(name="wpool", bufs=1))
    stage = ctx.enter_context(tc.tile_pool(name="stage", bufs=3))
    ptp = ctx.enter_context(tc.tile_pool(name="ptp", bufs=3))
    dvp = ctx.enter_context(tc.tile_pool(name="dvp", bufs=3))
    pacc = ctx.enter_context(tc.tile_pool(name="pacc", bufs=1, space="PSUM"))
    pst = ctx.enter_context(tc.tile_pool(name="pst", bufs=3, space="PSUM"))

    # ---- constants ----
    ones_dr = const.tile([P, 2, P], F8E4, tag="ones_dr", name="ones_dr")
    nc.vector.memset(ones_dr, 1.0)
    ones_row_bf = const.tile([1, P], BF16, tag="ones_row_bf", name="ones_row_bf")
    nc.vector.memset(ones_row_bf, 1.0)
    negshift = const.tile([P, 1], F32, tag="negshift", name="negshift")
    nc.vector.memset(negshift, -SHIFT)

    # ---- small inputs (biases host-duplicated to [128,1]) ----
    bq_sb = const.tile([P, 1], F32, tag="bq", name="bq_sb")
    nc.sync.dma_start(bq_sb, bq_d)
    bk_sb = const.tile([P, 1], F32, tag="bk", name="bk_sb")
    nc.sync.dma_start(bk_sb, bk_d)
    bv_sb = const.tile([1, C], F32, tag="bv", name="bv_sb")
    nc.sync.dma_start(bv_sb, bv_d)
    bv_bf = const.tile([1, C], BF16, tag="bvbf", name="bv_bf")
    nc.vector.tensor_copy(bv_bf, bv_sb)
    gamma_b = const.tile([P, 1], F32, tag="gamma_b", name="gamma_b")
    nc.sync.dma_start(gamma_b, gamma_d)

    # bv broadcast to all partitions once: [128, C] bf16
    bvb_ps = pst.tile([P, C], F32, tag="st", name="bvb_ps")
    nc.tensor.matmul(bvb_ps, ones_row_bf, bv_bf, start=True, stop=True)
    bv_bcast = const.tile([P, C], BF16, tag="bv_bcast", name="bv_bcast")
    nc.vector.tensor_copy(bv_bcast, bvb_ps)

    # ---- weights: host pre-transposed (+ dq-duplicated for Q/K); stage f32
    # then x16-scale into fp8e4 pair-sliceable tiles ----
    wq8 = wpool.tile([P, NCC, P], F8E4, tag="wq8", name="wq8")
    wk8 = wpool.tile([P, NCC, P], F8E4, tag="wk8", name="wk8")
    for j in range(NCC):
        for (src_d, dst) in ((wq_d, wq8), (wk_d, wk8)):
            wst = stage.tile([P, P], F32, tag="w_stage", name="w_st")
            nc.sync.dma_start(wst, src_d[ts(j, P), :])
            nc.vector.tensor_scalar_mul(dst[:, j, :], wst, WS)
    wv8 = wpool.tile([P, NCC, C], F8E4, tag="wv8", name="wv8")
    for j in range(NCC):
        wst2 = stage.tile([P, C], F32, tag="w_stage2", name="w_st2")
        nc.sync.dma_start(wst2, wv_d[ts(j, P), :])
        nc.vector.tensor_scalar_mul(wv8[:, j, :], wst2, WS)

    # ---- x resident fp8e4; gpsimd DMA casts f32->fp8 in flight.
    # xq piece 0 first (unblocks Q0), then all xkv, then the rest of xq.
    xq8 = persist.tile([P, NCC, N], F8E4, tag="xq8", name="xq8")
    xkv8 = persist.tile([P, NCC, N], F8E4, tag="xkv8", name="xkv8")
    for cc in range(NCC):
        nc.gpsimd.dma_start(xq8[:, cc, ts(0, 1024)], xq_d[ts(cc, P), ts(0, 1024)])
    # piece 0 of xkv in 512-col sub-chunks: K0 (cols 0-511) unlocks sooner
    for s in range(2):
        for cc in range(NCC):
            nc.gpsimd.dma_start(xkv8[:, cc, ts(s, 512)],
                                xkv_d[ts(cc, P), ts(s, 512)])
    for h in range(1, 4):  # remaining 1024-col pieces
        for cc in range(NCC):
            nc.gpsimd.dma_start(xkv8[:, cc, ts(h, 1024)],
                                xkv_d[ts(cc, P), ts(h, 1024)])
    for h in range(1, 4):
        for cc in range(NCC):
            nc.gpsimd.dma_start(xq8[:, cc, ts(h, 1024)],
                                xq_d[ts(cc, P), ts(h, 1024)])

    # ---- projection helpers (fp8 DoubleRow, contraction 2x128/pass) ----
    # Q/K land duplicated on both partition halves: [128(2x dq), N]
    Q_sb = persist.tile([P, N], BF16, tag="Q", name="Q_sb")
    K_sb = persist.tile([P, N], BF16, tag="K", name="K_sb")
    VT_sb = persist.tile([P, NKT, C], F8E4, tag="VT", name="VT_sb")

    def emit_k(nb, pool, tag):
        kp = pool.tile([P, QB], F32, tag=tag, name="k_ps")
        for m in range(NCC // 2):
            nc.tensor.matmul(kp, wk8[:, 2 * m:2 * m + 2, :],
                             xkv8[:, 2 * m:2 * m + 2, ts(nb, QB)],
                             start=(m == 0), stop=(m == 1), perf_mode=DR)
        nc.scalar.activation(K_sb[:, ts(nb, QB)], kp, AF.Identity,
                             bias=bk_sb, scale=1.0 / WS)

    def emit_q(nb, pool, tag):
        qp = pool.tile([P, QB], F32, tag=tag, name="q_ps")
        for m in range(NCC // 2):
            nc.tensor.matmul(qp, wq8[:, 2 * m:2 * m + 2, :],
                             xq8[:, 2 * m:2 * m + 2, ts(nb, QB)],
                             start=(m == 0), stop=(m == 1), perf_mode=DR)
        nc.scalar.activation(Q_sb[:, ts(nb, QB)], qp, AF.Identity,
                             bias=bq_sb, scale=1.0 / WS)

    def emit_vt(nt):
        vp = pst.tile([P, C], F32, tag="st", name="v_ps")
        for m in range(NCC // 2):
            nc.tensor.matmul(vp, xkv8[:, 2 * m:2 * m + 2, ts(nt, P)],
                             wv8[:, 2 * m:2 * m + 2, :],
                             start=(m == 0), stop=(m == 1), perf_mode=DR)
        nc.vector.scalar_tensor_tensor(VT_sb[:, nt, :], vp, 1.0 / WS, bv_bcast,
                                       op0=ALU.mult, op1=ALU.add)

    # K0/K1 (gated by xkv piece 0) and Q0/Q1 (xq piece 0) upfront in the
    # not-yet-rotating accumulator banks; everything else pipelines into
    # qblock 0's pair loop below.
    emit_k(0, pacc, "a0")
    emit_k(1, pacc, "a1")
    emit_q(0, pacc, "a2")
    emit_q(1, pacc, "a3")
    emit_vt(0)
    emit_vt(1)

    # ---- attention main loop ----
    def emit_st_exp(qb, t):
        """ST pair t (two row-tiled concurrent matmuls) + exp -> fp8e5 pair."""
        st0 = pst.tile([P, QB], F32, tag="st", name="st_ps")
        nc.tensor.matmul(st0, K_sb[0:DQ, ts(2 * t, P)],
                         Q_sb[0:DQ, ts(qb, QB)], start=True, stop=True)
        st1 = pst.tile([P, QB], F32, tag="st", name="st_ps")
        nc.tensor.matmul(st1, K_sb[DQ:P, ts(2 * t + 1, P)],
                         Q_sb[DQ:P, ts(qb, QB)], start=True, stop=True)
        pt = ptp.tile([P, 2, QB], F8E5, tag="pt", name="pt_sb", bufs=8)
        nc.scalar.activation(pt[:, 0, :], st0, AF.Exp, bias=negshift)
        nc.scalar.activation(pt[:, 1, :], st1, AF.Exp, bias=negshift)
        return pt

    def tail(qb, o_ps, rs_ps, xrs, last=False):
        # recip first (frees the rs bank). Mid-kernel: all four bank-freeing
        # muls before the residual stts (next qblock's PVs want the banks).
        # Last qblock: interleave per-cc so the final stores start earliest.
        recip_b = dvp.tile([P, QB], F32, tag="recip_b", name="recip_b")
        nc.vector.reciprocal_approx_fast(recip_b, rs_ps)
        t1s = []
        for cc in range(NCC):
            t1 = dvp.tile([P, QB], F32, tag=f"t1_{cc}", name="t1", bufs=2)
            nc.vector.tensor_mul(t1, o_ps[cc], recip_b)
            t1s.append(t1)
            if last:
                og = dvp.tile([P, QB], F32, tag="og", name="og", bufs=2)
                nc.vector.scalar_tensor_tensor(og, t1, gamma_b, xrs[cc],
                                               op0=ALU.mult, op1=ALU.add)
                nc.sync.dma_start(out_d[ts(cc, P), ts(qb, QB)], og)
        if not last:
            for cc in range(NCC):
                og = dvp.tile([P, QB], F32, tag="og", name="og", bufs=2)
                nc.vector.scalar_tensor_tensor(og, t1s[cc], gamma_b, xrs[cc],
                                               op0=ALU.mult, op1=ALU.add)
                nc.sync.dma_start(out_d[ts(cc, P), ts(qb, QB)], og)

    pt_next = emit_st_exp(0, 0)
    for qb in range(NQB):
        # fixed bank roles: rs reuses the bank the previous recip freed
        # (earliest release), o_cc the bank the cc-th normalization mul freed.
        o_ps = [pacc.tile([P, QB], F32, tag=f"a{cc}",
                          name=f"o_ps{cc}") for cc in range(NCC)]
        rs_ps = pacc.tile([P, QB], F32, tag="a4", name="rs_ps")
        # prefetch this qblock's residual slices (consumed by the tail)
        xrs = []
        for cc in range(NCC):
            xr = stage.tile([P, QB], F32, tag=f"xres{cc}", name="x_res", bufs=2)
            nc.sync.dma_start(xr, xq_d[ts(cc, P), ts(qb, QB)])
            xrs.append(xr)
        for t in range(NP):
            if qb == 0:
                # software-pipelined projections: VT pair one pair ahead of
                # its consumer, K block two pairs ahead, all through the st
                # psum ring.
                if t < NP - 1:
                    emit_vt(2 * t + 2)
                    emit_vt(2 * t + 3)
                if t % 2 == 1 and 3 <= t <= 13:
                    emit_k((t + 1) // 2, pst, "st")
            pt = pt_next
            g = qb * NP + t + 1
            if g < NQB * NP:
                pt_next = emit_st_exp(g // NP, g % NP)
            # rs first: at pair 15 its stop unblocks the recip early, and at
            # pair 0 it reuses the earliest-freed (recip) bank.
            nc.tensor.matmul(rs_ps, ones_dr, pt,
                             start=(t == 0), stop=(t == NP - 1), perf_mode=DR)
            for cc in range(NCC):
                nc.tensor.matmul(o_ps[cc], VT_sb[:, 2 * t:2 * t + 2, ts(cc, P)],
                                 pt, start=(t == 0), stop=(t == NP - 1),
                                 perf_mode=DR)
            if t == 8 and qb + 2 < NQB:
                emit_q(qb + 2, pst, "st")
        tail(qb, o_ps, rs_ps, xrs, last=(qb == NQB - 1))


_NC_CACHE = {}


def _fuse_ldweights(nc):
    """Re-fuse Tile's split LDWEIGHTS+MATMUL pairs into self-loading matmuls
    so walrus's ldw-opt (background weight buffer) can overlap weight loads
    with in-flight matmuls."""
    for b in nc.m.functions[0].blocks:
        out = []
        pending = None
        for i in b.instructions:
            tn = type(i).__name__
            if tn == "InstLdweights":
                assert pending is None, "back-to-back ldweights"
                pending = i
                continue
            if tn == "InstMatmult" and pending is not None:
                i.ldweights = True
                si = pending.sync_info
                if si is not None and (si.on_wait or si.on_update):
                    if i.sync_info is None:
                        i.sync_info = mybir.SyncInfo(on_wait=[], on_update=[])
                    i.sync_info.on_wait = list(si.on_wait) + list(i.sync_info.on_wait)
                    i.sync_info.on_update = (list(si.on_update)
                                             + list(i.sync_info.on_update))
                pending = None
            out.append(i)
        assert pending is None, "trailing ldweights without matmul"
        b.instructions[:] = out


def _build():
    if "nc" in _NC_CACHE:
        return _NC_CACHE["nc"]
    nc = bacc.Bacc("TRN2", target_bir_lowering=False, debug=False, num_devices=8)
    io = {
        "xq": nc.dram_tensor("xq", [C, N], F32, kind="ExternalInput").ap(),
        "xkv": nc.dram_tensor("xkv", [C, N], F32, kind="ExternalInput").ap(),
        "wq": nc.dram_tensor("wq", [C, P], F32, kind="ExternalInput").ap(),
        "wk": nc.dram_tensor("wk", [C, P], F32, kind="ExternalInput").ap(),
        "wv": nc.dram_tensor("wv", [C, C], F32, kind="ExternalInput").ap(),
        "bq": nc.dram_tensor("bq", [P, 1], F32, kind="ExternalInput").ap(),
        "bk": nc.dram_tensor("bk", [P, 1], F32, kind="ExternalInput").ap(),
        "bv": nc.dram_tensor("bv", [1, C], F32, kind="ExternalInput").ap(),
        "gamma": nc.dram_tensor("gamma", [P, 1], F32, kind="ExternalInput").ap(),
        "out": nc.dram_tensor("out", [C, N], F32, kind="ExternalOutput").ap(),
    }
    with tile.TileContext(nc) as tc:
        _body(tc, io)
    _fuse_ldweights(nc)
    nc.compile()
    _NC_CACHE["nc"] = nc
    return nc


def make_in_maps(x1, x2, wq1, bq1, wk1, bk1, wv1, bv1,
                 wq2, bq2, wk2, bk2, wv2, bv2, gamma1, gamma2):
    """Returns the 8 per-core input dicts. Cores 0-3: out1[b]; 4-7: out2[b]."""
    f = np.ascontiguousarray
    x1f = np.asarray(x1, np.float32).reshape(B, C, N)
    x2f = np.asarray(x2, np.float32).reshape(B, C, N)

    def wdup(w):  # [DQ, C] -> [C, 2*DQ] (transposed, duplicated)
        wt = np.asarray(w, np.float32).T
        return f(np.concatenate([wt, wt], axis=1))

    def bdup(b):  # [DQ] -> [128, 1]
        bb = np.asarray(b, np.float32).reshape(DQ, 1)
        return f(np.concatenate([bb, bb], axis=0))

    maps = []
    for b in range(B):
        maps.append({
            "xq": f(x1f[b]), "xkv": f(x2f[b]),
            "wq": wdup(wq1), "wk": wdup(wk2),
            "wv": f(np.asarray(wv2, np.float32).T),
            "bq": bdup(bq1), "bk": bdup(bk2),
            "bv": f(np.asarray(bv2, np.float32).reshape(1, C)),
            "gamma": f(np.tile(np.asarray(gamma1, np.float32).reshape(1, 1), (P, 1))),
        })
    for b in range(B):
        maps.append({
            "xq": f(x2f[b]), "xkv": f(x1f[b]),
            "wq": wdup(wq2), "wk": wdup(wk1),
            "wv": f(np.asarray(wv1, np.float32).T),
            "bq": bdup(bq2), "bk": bdup(bk1),
            "bv": f(np.asarray(bv1, np.float32).reshape(1, C)),
            "gamma": f(np.tile(np.asarray(gamma2, np.float32).reshape(1, 1), (P, 1))),
        })
    return maps


def kernel(**inputs):
    nc = _build()
    in_maps = make_in_maps(**inputs)
    res = run_bass_kernel_spmd(nc, in_maps, list(range(8))).results
    out1 = np.stack([res[b]["out"].reshape(C, H, W) for b in range(B)])
    out2 = np.stack([res[B + b]["out"].reshape(C, H, W) for b in range(B)])
    return out1, out2


# revision 19
# speedup vs baseline: 1.2027x; 1.0056x over previous
"""Trainium2 Bass kernel for dual cross-attention (CotSR block).

Problem: two cross-attentions between x1, x2 [B=4, C=512, H=W=64].
  q1 = wq1@x1, k2 = wk2@x2, v2 = wv2@x2 ; att1 = softmax(q1^T k2) over keys
  out1 = x1 + gamma1 * (v2 @ att1^T)   (and symmetrically for out2)

Sharding: 8 independent (batch, direction) jobs -> one per NeuronCore.

Per-core dataflow (N = 4096 tokens, DQ = 64, C = 512), v4:
  - x arrives as fp8e4 (gpsimd DMA casts f32->fp8 in flight); xq piece 0
    first so Q0 unblocks early.
  - All projections are fp8 DoubleRow matmuls (contraction 256/pass);
    weights host-transposed, x16-scaled into fp8e4; evictions scale 1/16.
    Q/K weights host-duplicated so dq=64 lands on partitions 0-63 AND
    64-127 (enables 2x row-tiled ST matmuls).
  - K blocks and VT tiles are software-pipelined INTO qblock 0's pair loop
    (pair t only needs K/VT keytiles 2t,2t+1), so attention starts as soon
    as the first xkv DMA piece lands instead of after all projections.
  - Attention per 512-query block, over 16 key-tile PAIRS (256 keys each):
      ST:  two concurrent row-tiled bf16 matmuls (rows 0-63 / 64-127)
      PT:  exp(ST - 7) -> fp8e5 pair tile [128, 2, 512]  (global shift is
           exact for softmax: numerator and denominator both scale e^-7)
      RS:  1 DoubleRow matmul, all-ones lhsT -> BROADCAST row-sum [128,512]
      PV:  4 DoubleRow fp8 matmuls (contraction 256) accumulate [128c, 512q]
  - PSUM 5-bank accumulator rotation: qblock qb+1's first PV reuses the
    bank freed by qb's recip (earliest), later PVs the banks freed by the
    per-cc normalization muls -> near-zero qblock seam.
  PSUM: 5 banks rotating O/RS + 3 banks ST ring = 8.
"""

import numpy as np

import concourse.bass as bass
import concourse.mybir as mybir
import concourse.tile as tile
from concourse import bacc
import concourse.bass_utils as _bu

# walrus's --enable-ldw-opt=false serializes every LDWEIGHTS with its MATMUL
# (measured 379 ns/MM vs ~215 warm); enable background-weight-buffer overlap.
_orig_run_command = _bu.run_command


def _patched_run_command(argv, **kw):
    argv = ["--enable-ldw-opt=true" if a == "--enable-ldw-opt=false" else a
            for a in argv]
    return _orig_run_command(argv, **kw)


_bu.run_command = _patched_run_command
from concourse.bass_utils import run_bass_kernel_spmd
from concourse._compat import with_exitstack
from contextlib import ExitStack

F32 = mybir.dt.float32
BF16 = mybir.dt.bfloat16
F8E4 = mybir.dt.float8e4
F8E5 = mybir.dt.float8e5
AF = mybir.ActivationFunctionType
ALU = mybir.AluOpType
DR = mybir.MatmulPerfMode.DoubleRow
ts = bass.ts

B, C, H, W = 4, 512, 64, 64
N = H * W          # 4096
DQ = 64
P = 128
QB = 512           # query block (free dim of ST / moving operand)
NQB = N // QB      # 8 query blocks
NKT = N // P       # 32 key tiles
NP = NKT // 2      # 16 key-tile pairs
NCC = C // P       # 4 channel chunks
SHIFT = 7.0        # global logit shift before exp (softmax-invariant)
WS = 16.0          # fp8 weight scale (undone at psum eviction)


@with_exitstack
def _body(ctx: ExitStack, tc: "tile.TileContext", io: dict):
    nc = tc.nc
    xq_d, xkv_d, wq_d, wk_d, wv_d = io["xq"], io["xkv"], io["wq"], io["wk"], io["wv"]
    bq_d, bk_d, bv_d, gamma_d, out_d = io["bq"], io["bk"], io["bv"], io["gamma"], io["out"]

    const = ctx.enter_context(tc.tile_pool(name="const", bufs=1))
    persist = ctx.enter_context(tc.tile_pool(name="persist", bufs=1))
    wpool = ctx.enter_context(tc.tile_pool(name="wpool", bufs=1))
    stage = ctx.enter_context(tc.tile_pool(name="stage", bufs=3))
    ptp = ctx.enter_context(tc.tile_pool(name="ptp", bufs=3))
    dvp = ctx.enter_context(tc.tile_pool(name="dvp", bufs=3))
    pacc = ctx.enter_context(tc.tile_pool(name="pacc", bufs=1, space="PSUM"))
    pst = ctx.enter_context(tc.tile_pool(name="pst", bufs=3, space="PSUM"))

    # ---- constants ----
    ones_dr = const.tile([P, 2, P], F8E4, tag="ones_dr", name="ones_dr")
    nc.vector.memset(ones_dr, 1.0)
    ones_row_bf = const.tile([1, P], BF16, tag="ones_row_bf", name="ones_row_bf")
    nc.vector.memset(ones_row_bf, 1.0)
    negshift = const.tile([P, 1], F32, tag="negshift", name="negshift")
    nc.vector.memset(negshift, -SHIFT)

    # ---- small inputs (biases host-duplicated to [128,1]) ----
    bq_sb = const.tile([P, 1], F32, tag="bq", name="bq_sb")
    nc.sync.dma_start(bq_sb, bq_d)
    bk_sb = const.tile([P, 1], F32, tag="bk", name="bk_sb")
    nc.sync.dma_start(bk_sb, bk_d)
    bv_sb = const.tile([1, C], F32, tag="bv", name="bv_sb")
    nc.sync.dma_start(bv_sb, bv_d)
    bv_bf = const.tile([1, C], BF16, tag="bvbf", name="bv_bf")
    nc.vector.tensor_copy(bv_bf, bv_sb)
    gamma_b = const.tile([P, 1], F32, tag="gamma_b", name="gamma_b")
    nc.sync.dma_start(gamma_b, gamma_d)

    # bv broadcast to all partitions once: [128, C] bf16
    bvb_ps = pst.tile([P, C], F32, tag="st", name="bvb_ps")
    nc.tensor.matmul(bvb_ps, ones_row_bf, bv_bf, start=True, stop=True)
    bv_bcast = const.tile([P, C], BF16, tag="bv_bcast", name="bv_bcast")
    nc.vector.tensor_copy(bv_bcast, bvb_ps)

    # ---- weights: host pre-transposed (+ dq-duplicated for Q/K); stage f32
    # then x16-scale into fp8e4 pair-sliceable tiles ----
    wq8 = wpool.tile([P, NCC, P], F8E4, tag="wq8", name="wq8")
    wk8 = wpool.tile([P, NCC, P], F8E4, tag="wk8", name="wk8")
    for j in range(NCC):
        for (src_d, dst) in ((wq_d, wq8), (wk_d, wk8)):
            wst = stage.tile([P, P], F32, tag="w_stage", name="w_st")
            nc.sync.dma_start(wst, src_d[ts(j, P), :])
            nc.vector.tensor_scalar_mul(dst[:, j, :], wst, WS)
    wv8 = wpool.tile([P, NCC, C], F8E4, tag="wv8", name="wv8")
    for j in range(NCC):
        wst2 = stage.tile([P, C], F32, tag="w_stage2", name="w_st2")
        nc.sync.dma_start(wst2, wv_d[ts(j, P), :])
        nc.vector.tensor_scalar_mul(wv8[:, j, :], wst2, WS)

    # ---- x resident fp8e4; gpsimd DMA casts f32->fp8 in flight.
    # xq piece 0 first (unblocks Q0), then all xkv, then the rest of xq.
    xq8 = persist.tile([P, NCC, N], F8E4, tag="xq8", name="xq8")
    xkv8 = persist.tile([P, NCC, N], F8E4, tag="xkv8", name="xkv8")
    # only the 512 columns Q0 needs upfront; Q1's columns follow xkv piece 1
    for cc in range(NCC):
        nc.gpsimd.dma_start(xq8[:, cc, ts(0, 512)], xq_d[ts(cc, P), ts(0, 512)])
    # piece 0 of xkv in 512-col sub-chunks: K0 (cols 0-511) unlocks sooner
    for s in range(2):
        for cc in range(NCC):
            nc.gpsimd.dma_start(xkv8[:, cc, ts(s, 512)],
                                xkv_d[ts(cc, P), ts(s, 512)])
    for cc in range(NCC):
        nc.gpsimd.dma_start(xkv8[:, cc, ts(1, 1024)],
                            xkv_d[ts(cc, P), ts(1, 1024)])
    for cc in range(NCC):
        nc.gpsimd.dma_start(xq8[:, cc, ts(1, 512)], xq_d[ts(cc, P), ts(1, 512)])
    for h in range(2, 4):
        for cc in range(NCC):
            nc.gpsimd.dma_start(xkv8[:, cc, ts(h, 1024)],
                                xkv_d[ts(cc, P), ts(h, 1024)])
    for h in range(1, 4):
        for cc in range(NCC):
            nc.gpsimd.dma_start(xq8[:, cc, ts(h, 1024)],
                                xq_d[ts(cc, P), ts(h, 1024)])

    # ---- projection helpers (fp8 DoubleRow, contraction 2x128/pass) ----
    # Q/K land duplicated on both partition halves: [128(2x dq), N]
    Q_sb = persist.tile([P, N], BF16, tag="Q", name="Q_sb")
    K_sb = persist.tile([P, N], BF16, tag="K", name="K_sb")
    VT_sb = persist.tile([P, NKT, C], F8E4, tag="VT", name="VT_sb")

    def emit_k(nb, pool, tag):
        kp = pool.tile([P, QB], F32, tag=tag, name="k_ps")
        for m in range(NCC // 2):
            nc.tensor.matmul(kp, wk8[:, 2 * m:2 * m + 2, :],
                             xkv8[:, 2 * m:2 * m + 2, ts(nb, QB)],
                             start=(m == 0), stop=(m == 1), perf_mode=DR)
        nc.scalar.activation(K_sb[:, ts(nb, QB)], kp, AF.Identity,
                             bias=bk_sb, scale=1.0 / WS)

    def emit_q(nb, pool, tag):
        qp = pool.tile([P, QB], F32, tag=tag, name="q_ps")
        for m in range(NCC // 2):
            nc.tensor.matmul(qp, wq8[:, 2 * m:2 * m + 2, :],
                             xq8[:, 2 * m:2 * m + 2, ts(nb, QB)],
                             start=(m == 0), stop=(m == 1), perf_mode=DR)
        nc.scalar.activation(Q_sb[:, ts(nb, QB)], qp, AF.Identity,
                             bias=bq_sb, scale=1.0 / WS)

    def emit_vt(nt):
        vp = pst.tile([P, C], F32, tag="st", name="v_ps")
        for m in range(NCC // 2):
            nc.tensor.matmul(vp, xkv8[:, 2 * m:2 * m + 2, ts(nt, P)],
                             wv8[:, 2 * m:2 * m + 2, :],
                             start=(m == 0), stop=(m == 1), perf_mode=DR)
        nc.vector.scalar_tensor_tensor(VT_sb[:, nt, :], vp, 1.0 / WS, bv_bcast,
                                       op0=ALU.mult, op1=ALU.add)

    # K0/K1 (gated by xkv piece 0) and Q0/Q1 (xq piece 0) upfront in the
    # not-yet-rotating accumulator banks; everything else pipelines into
    # qblock 0's pair loop below.
    emit_k(0, pacc, "a0")
    emit_k(1, pacc, "a1")
    emit_q(0, pacc, "a2")
    emit_vt(0)
    emit_vt(1)

    # ---- attention main loop ----
    def emit_st_exp(qb, t):
        """ST pair t (two row-tiled concurrent matmuls) + exp -> fp8e5 pair."""
        st0 = pst.tile([P, QB], F32, tag="st", name="st_ps")
        nc.tensor.matmul(st0, K_sb[0:DQ, ts(2 * t, P)],
                         Q_sb[0:DQ, ts(qb, QB)], start=True, stop=True)
        st1 = pst.tile([P, QB], F32, tag="st", name="st_ps")
        nc.tensor.matmul(st1, K_sb[DQ:P, ts(2 * t + 1, P)],
                         Q_sb[DQ:P, ts(qb, QB)], start=True, stop=True)
        pt = ptp.tile([P, 2, QB], F8E5, tag="pt", name="pt_sb", bufs=8)
        nc.scalar.activation(pt[:, 0, :], st0, AF.Exp, bias=negshift)
        nc.scalar.activation(pt[:, 1, :], st1, AF.Exp, bias=negshift)
        return pt

    def tail(qb, o_ps, rs_ps, xrs, last=False):
        # recip first (frees the rs bank). Mid-kernel: all four bank-freeing
        # muls before the residual stts (next qblock's PVs want the banks).
        # Last qblock: interleave per-cc so the final stores start earliest.
        recip_b = dvp.tile([P, QB], F32, tag="recip_b", name="recip_b")
        nc.vector.reciprocal_approx_fast(recip_b, rs_ps)
        t1s = []
        for cc in range(NCC):
            t1 = dvp.tile([P, QB], F32, tag=f"t1_{cc}", name="t1", bufs=2)
            nc.vector.tensor_mul(t1, o_ps[cc], recip_b)
            t1s.append(t1)
            if last:
                og = dvp.tile([P, QB], F32, tag="og", name="og", bufs=2)
                nc.vector.scalar_tensor_tensor(og, t1, gamma_b, xrs[cc],
                                               op0=ALU.mult, op1=ALU.add)
                nc.sync.dma_start(out_d[ts(cc, P), ts(qb, QB)], og)
        if not last:
            for cc in range(NCC):
                og = dvp.tile([P, QB], F32, tag="og", name="og", bufs=2)
                nc.vector.scalar_tensor_tensor(og, t1s[cc], gamma_b, xrs[cc],
                                               op0=ALU.mult, op1=ALU.add)
                nc.sync.dma_start(out_d[ts(cc, P), ts(qb, QB)], og)

    pt_next = emit_st_exp(0, 0)
    for qb in range(NQB):
        # fixed bank roles: rs reuses the bank the previous recip freed
        # (earliest release), o_cc the bank the cc-th normalization mul freed.
        o_ps = [pacc.tile([P, QB], F32, tag=f"a{cc}",
                          name=f"o_ps{cc}") for cc in range(NCC)]
        rs_ps = pacc.tile([P, QB], F32, tag="a4", name="rs_ps")
        # prefetch this qblock's residual slices (consumed by the tail)
        xrs = []
        for cc in range(NCC):
            xr = stage.tile([P, QB], F32, tag=f"xres{cc}", name="x_res", bufs=2)
            nc.sync.dma_start(xr, xq_d[ts(cc, P), ts(qb, QB)])
            xrs.append(xr)
        for t in range(NP):
            if qb == 0:
                # software-pipelined projections: VT pair one pair ahead of
                # its consumer, K block two pairs ahead, all through the st
                # psum ring.
                if t < NP - 1:
                    emit_vt(2 * t + 2)
                    emit_vt(2 * t + 3)
                if t % 2 == 1 and 3 <= t <= 13:
                    emit_k((t + 1) // 2, pst, "st")
                if t == 6:
                    emit_q(1, pst, "st")
            pt = pt_next
            g = qb * NP + t + 1
            if g < NQB * NP:
                pt_next = emit_st_exp(g // NP, g % NP)
            # rs mid-pair: its 256-col ldweights hides under PV1's matmul.
            # At pair 15 rs goes first so its stop unblocks the recip early.
            if t == NP - 1:
                nc.tensor.matmul(rs_ps, ones_dr, pt,
                                 start=False, stop=True, perf_mode=DR)
            for cc in range(NCC):
                nc.tensor.matmul(o_ps[cc], VT_sb[:, 2 * t:2 * t + 2, ts(cc, P)],
                                 pt, start=(t == 0), stop=(t == NP - 1),
                                 perf_mode=DR)
                if cc == 1 and t < NP - 1:
                    nc.tensor.matmul(rs_ps, ones_dr, pt,
                                     start=(t == 0), stop=False, perf_mode=DR)
            if t == 8 and qb + 2 < NQB:
                emit_q(qb + 2, pst, "st")
        tail(qb, o_ps, rs_ps, xrs, last=(qb == NQB - 1))


_NC_CACHE = {}


def _fuse_ldweights(nc):
    """Re-fuse Tile's split LDWEIGHTS+MATMUL pairs into self-loading matmuls
    so walrus's ldw-opt (background weight buffer) can overlap weight loads
    with in-flight matmuls."""
    for b in nc.m.functions[0].blocks:
        out = []
        pending = None
        for i in b.instructions:
            tn = type(i).__name__
            if tn == "InstLdweights":
                assert pending is None, "back-to-back ldweights"
                pending = i
                continue
            if tn == "InstMatmult" and pending is not None:
                i.ldweights = True
                si = pending.sync_info
                if si is not None and (si.on_wait or si.on_update):
                    if i.sync_info is None:
                        i.sync_info = mybir.SyncInfo(on_wait=[], on_update=[])
                    i.sync_info.on_wait = list(si.on_wait) + list(i.sync_info.on_wait)
                    i.sync_info.on_update = (list(si.on_update)
                                             + list(i.sync_info.on_update))
                pending = None
            out.append(i)
        assert pending is None, "trailing ldweights without matmul"
        b.instructions[:] = out


def _build():
    if "nc" in _NC_CACHE:
        return _NC_CACHE["nc"]
    nc = bacc.Bacc("TRN2", target_bir_lowering=False, debug=False, num_devices=8)
    io = {
        "xq": nc.dram_tensor("xq", [C, N], F32, kind="ExternalInput").ap(),
        "xkv": nc.dram_tensor("xkv", [C, N], F32, kind="ExternalInput").ap(),
        "wq": nc.dram_tensor("wq", [C, P], F32, kind="ExternalInput").ap(),
        "wk": nc.dram_tensor("wk", [C, P], F32, kind="ExternalInput").ap(),
        "wv": nc.dram_tensor("wv", [C, C], F32, kind="ExternalInput").ap(),
        "bq": nc.dram_tensor("bq", [P, 1], F32, kind="ExternalInput").ap(),
        "bk": nc.dram_tensor("bk", [P, 1], F32, kind="ExternalInput").ap(),
        "bv": nc.dram_tensor("bv", [1, C], F32, kind="ExternalInput").ap(),
        "gamma": nc.dram_tensor("gamma", [P, 1], F32, kind="ExternalInput").ap(),
        "out": nc.dram_tensor("out", [C, N], F32, kind="ExternalOutput").ap(),
    }
    with tile.TileContext(nc) as tc:
        _body(tc, io)
    _fuse_ldweights(nc)
    nc.compile()
    _NC_CACHE["nc"] = nc
    return nc


def make_in_maps(x1, x2, wq1, bq1, wk1, bk1, wv1, bv1,
                 wq2, bq2, wk2, bk2, wv2, bv2, gamma1, gamma2):
    """Returns the 8 per-core input dicts. Cores 0-3: out1[b]; 4-7: out2[b]."""
    f = np.ascontiguousarray
    x1f = np.asarray(x1, np.float32).reshape(B, C, N)
    x2f = np.asarray(x2, np.float32).reshape(B, C, N)

    def wdup(w):  # [DQ, C] -> [C, 2*DQ] (transposed, duplicated)
        wt = np.asarray(w, np.float32).T
        return f(np.concatenate([wt, wt], axis=1))

    def bdup(b):  # [DQ] -> [128, 1]
        bb = np.asarray(b, np.float32).reshape(DQ, 1)
        return f(np.concatenate([bb, bb], axis=0))

    maps = []
    for b in range(B):
        maps.append({
            "xq": f(x1f[b]), "xkv": f(x2f[b]),
            "wq": wdup(wq1), "wk": wdup(wk2),
            "wv": f(np.asarray(wv2, np.float32).T),
            "bq": bdup(bq1), "bk": bdup(bk2),
            "bv": f(np.asarray(bv2, np.float32).reshape(1, C)),
            "gamma": f(np.tile(np.asarray(gamma1, np.float32).reshape(1, 1), (P, 1))),
        })
    for b in range(B):
        maps.append({
            "xq": f(x2f[b]), "xkv": f(x1f[b]),
            "wq": wdup(wq2), "wk": wdup(wk1),
            "wv": f(np.asarray(wv1, np.float32).T),
            "bq": bdup(bq2), "bk": bdup(bk1),
            "bv": f(np.asarray(bv1, np.float32).reshape(1, C)),
            "gamma": f(np.tile(np.asarray(gamma2, np.float32).reshape(1, 1), (P, 1))),
        })
    return maps


def kernel(**inputs):
    nc = _build()
    in_maps = make_in_maps(**inputs)
    res = run_bass_kernel_spmd(nc, in_maps, list(range(8))).results
    out1 = np.stack([res[b]["out"].reshape(C, H, W) for b in range(B)])
    out2 = np.stack([res[B + b]["out"].reshape(C, H, W) for b in range(B)])
    return out1, out2
